# revision 1
# baseline (speedup 1.0000x reference)
"""Trainium2 Bass kernel: batched dense attention.

Full inputs: queries/keys/values [16, 2048, 64] fp32.
Shards batch dim across 8 NeuronCores (2 batches per core).

Per-core algorithm (batches A, B local):
  S^T[j, q] = K[j, :] . Q[q, :]           (PE, fp16 operands, fp32 PSUM)
  P^T = exp(S^T / 8)                       (ACT, PSUM->SBUF fp16, scale fused)
  O'^T[d', q] = sum_j V'[j, d'] P^T[j, q]  (PE; V' = [V | ones] so row 64 = softmax sums)
  O[q, :] = O'^T[0:64, q].T / O'^T[64, q]  (PE transpose + DVE reciprocal/mult)

Layout tricks:
  - Q^T / K^T built by PE transposes of natural tiles, batch-PAIRED so batch A
    lands on SBUF partitions 0-63 and batch B on 64-127.  QK^T matmuls for the
    two batches then row-pack on the PE (disjoint row groups, concurrent).
  - S^T PSUM tiles are grouped 3-wide ([128, 1536]) so each ACT exp instruction
    amortizes its per-instruction overhead; the ACT engine is the bottleneck
    (~64us busy of ~84us modeled total per core), so the whole schedule is
    arranged to keep it fed: 2 rotating 3-bank S^T buffers, loads chunked by
    need, prep transposes and PV chains placed so the pv-PSUM pool's FIFO
    allocation order matches data arrival.
  - All matmul operands are fp16 (1 cycle/row on the PE, ~5e-4 relative error
    for this data); PSUM accumulation stays fp32.
  - The softmax denominators ride along as a 65th "ones" column of V, and the
    final transpose packs all four [65,128] tiles of a q-block into one PSUM
    bank, divided by a single broadcast multiply.
"""

import sys
for _p in ("/opt/trn_rl_repo", "/root/.axon_site/_ro/trn_rl_repo"):
    if _p not in sys.path:
        sys.path.insert(0, _p)

import numpy as np

import concourse.bass as bass
import concourse.mybir as mybir
from concourse import bacc
from concourse.tile import TileContext
from concourse.masks import make_identity

F32 = mybir.dt.float32
F16 = mybir.dt.float16
P = 128

N_CORES = 8
B_FULL, N, D = 16, 2048, 64
B_LOC = B_FULL // N_CORES          # 2 batches per core
NT = N // P                        # 16 tiles of 128 along q and j
QB = 512                           # q-block (PSUM bank width in fp32)
NQB = N // QB                      # 4 q-blocks
GROUP = 3                          # S^T stream tiles per ACT exp instruction
N_STREAM = NQB * 2 * NT            # 128 S^T stream tiles per core
N_GROUPS = (N_STREAM + GROUP - 1) // GROUP

_nc_cache = None


def build():
    nc = bacc.Bacc(None, target_bir_lowering=False)
    q_hbm = nc.dram_tensor("queries", [B_LOC, N, D], F32, kind="ExternalInput")
    k_hbm = nc.dram_tensor("keys", [B_LOC, N, D], F32, kind="ExternalInput")
    v_hbm = nc.dram_tensor("values", [B_LOC, N, D], F32, kind="ExternalInput")
    o_hbm = nc.dram_tensor("out", [B_LOC, N, D], F32, kind="ExternalOutput")

    with TileContext(nc) as tc:
        with (
            tc.tile_pool(name="cst", bufs=1) as cst,
            tc.tile_pool(name="stage", bufs=2) as stage,
            tc.tile_pool(name="persist", bufs=1) as persist,
            tc.tile_pool(name="pt", bufs=24) as ptp,
            tc.tile_pool(name="otp", bufs=4) as otp,
            tc.tile_pool(name="ost", bufs=4) as ostp,
            tc.tile_pool(name="st", bufs=2, space="PSUM") as stp,
            tc.tile_pool(name="pv", bufs=2, space="PSUM") as pvp,
        ):
            ident = cst.tile([P, P], F16)
            make_identity(nc, ident)

            # ---- persistent SBUF buffers ----
            # Q^T / K^T, batch-paired: rows 0-63 batch A (d), 64-127 batch B.
            qt = persist.tile([P, N], F16, tag="qt")
            kt = persist.tile([P, N], F16, tag="kt")
            # V' = [V | ones]: [128 j, b, jt, 65] fp16
            v16 = persist.tile([P, B_LOC, NT, D + 1], F16, tag="v16")

            # ---- prep: load, cast, transpose ----
            # staging layout [128, t, (b d)]: per partition q, tile t, batch-major d
            q_res = [q_hbm[b, :, :].rearrange("(t p) d -> p t d", p=P)
                     for b in range(B_LOC)]
            k_res = [k_hbm[b, :, :].rearrange("(t p) d -> p t d", p=P)
                     for b in range(B_LOC)]

            # Loads: each dma_start costs ~650ns serial issue on the SP HWDGE
            # ring and the data phases serialize across the 16 SDMA engines, so
            # order chunks by when the pipeline needs them: K/Q tiles 0-3
            # first (first matmuls), then 4-7, then the rest, V before Q 8-15.
            st32s, st16s = {}, {}
            for name in ("k", "q"):
                st32s[name] = stage.tile([P, NT, B_LOC, D], F32, tag=f"{name}s32",
                                         name=f"{name}s32")
                st16s[name] = stage.tile([P, NT, B_LOC, D], F16, tag=f"{name}s16",
                                         name=f"{name}s16")
            vs32 = stage.tile([P, B_LOC, NT, D], F32, tag="vs32")

            def load_chunk(name, t0, t1, eng=None):
                re_aps = k_res if name == "k" else q_res
                cs = slice(t0, t1)
                for b in range(B_LOC):
                    (eng or nc.sync).dma_start(st32s[name][:, cs, b, :],
                                               re_aps[b][:, cs, :])
                nc.vector.tensor_copy(st16s[name][:, cs], st32s[name][:, cs])

            load_chunk("k", 0, 4)
            load_chunk("q", 0, 4)
            load_chunk("k", 4, 8)
            load_chunk("q", 4, 8)
            load_chunk("k", 8, NT)
            # V: [128, b, t, d] staging -> cast into v16[:, :, :, :64], ones col
            for b in range(B_LOC):
                v_re = v_hbm[b, :, :].rearrange("(t p) d -> p t d", p=P)
                nc.sync.dma_start(vs32[:, b], v_re)
            nc.vector.tensor_copy(v16[:, :, :, 0:D], vs32[:])
            nc.vector.memset(v16[:, :, :, D:D + 1], 1.0)
            load_chunk("q", 8, NT)

            def prep_transpose(name, t, dst, pool=None):
                """PE-transpose natural tile t of q/k into dst[:, t*128...]."""
                pool = pool or pvp
                tp_ps = pool.tile([P, P], F16, tag="st" if pool is stp else "pv",
                                  name=f"tp_{name}{t}")
                nc.tensor.transpose(tp_ps[:], st16s[name][:, t], ident[:])
                nc.vector.tensor_copy(dst[:, t * P:(t + 1) * P], tp_ps[:])

            # K^T tiles j0-1 now; the rest are emitted lazily inside the qb0
            # loop (pool allocations are FIFO in emission order, so a transpose
            # emitted before its staging data lands would stall the slot ring).
            for t in range(2):
                prep_transpose("k", t, kt, pool=(stp if t % 2 else pvp))

            # ---- main loop ----
            # stream of S^T tiles: s = qb*32 + 2*j + b
            st_tiles = {}      # group -> psum tile
            pt_tiles = {}      # group -> sbuf fp16 tile
            o_ps = {}          # b -> current PV accumulation psum tile

            # groups never span a q-block boundary: per qb there are 2*NT=32
            # stream tiles -> 10 groups of 3 + 1 group of 2.
            SPQ = 2 * NT                       # stream tiles per q-block
            GPQ = (SPQ + GROUP - 1) // GROUP   # groups per q-block

            def group_of(s):
                qb_, sl = divmod(s, SPQ)
                return qb_ * GPQ + sl // GROUP, sl % GROUP

            def tiles_in_group(g):
                return min(GROUP, SPQ - (g % GPQ) * GROUP)

            def maybe_exp(g):
                """emit exp for group g once all its stream tiles are written"""
                n_in_g = tiles_in_group(g)
                pt_t = ptp.tile([P, GROUP * QB], F16, tag="pt", name=f"ptg{g}")
                nc.scalar.activation(
                    pt_t[:, :n_in_g * QB], st_tiles[g][:, :n_in_g * QB],
                    mybir.ActivationFunctionType.Exp, scale=0.125,
                )
                pt_tiles[g] = pt_t

            # Q^T tiles for q-block 0
            QTPB = QB // P  # q-tiles per q-block
            for t in range(QTPB):
                prep_transpose("q", t, qt, pool=(stp if t % 2 else pvp))

            for qb in range(NQB):
                qs = slice(qb * QB, (qb + 1) * QB)
                # PV trails QK by PV_LAG steps.  qb0's pv-pool FIFO is occupied
                # by the lazy K transposes until step 13, so its PV starts at
                # step 14; later blocks keep PV after the full QK loop (the
                # scheduler back-fills PE gaps with it).
                PV_LAG = 14 if qb == 0 else 4
                for step in range(NT + PV_LAG):
                    if step < NT:
                        j = step
                        if qb == 0 and j + 2 < NT:
                            prep_transpose("k", j + 2, kt)
                        # next q-block's Q^T transposes early (before this
                        # block's o_ps claims the pv slots at PV_LAG)
                        tp_q_step = 8 if qb == 0 else 0
                        if qb + 1 < NQB and tp_q_step <= step < tp_q_step + 2:
                            for i in range(2):
                                prep_transpose("q", (qb + 1) * QTPB + 2 * (step - tp_q_step) + i, qt)
                        for b in range(B_LOC):
                            s = qb * SPQ + 2 * j + b
                            g, slot = group_of(s)
                            if slot == 0:
                                st_tiles[g] = stp.tile([P, GROUP * QB], F32, tag="st", name=f"stg{g}")
                            rows = slice(b * D, (b + 1) * D)
                            nc.tensor.matmul(
                                st_tiles[g][:, slot * QB:(slot + 1) * QB],
                                kt[rows, j * P:(j + 1) * P],
                                qt[rows, qs],
                                start=True, stop=True,
                            )
                            if slot == tiles_in_group(g) - 1:
                                maybe_exp(g)
                    if step == PV_LAG:
                        for b in range(B_LOC):
                            o_ps[b] = pvp.tile([D + 1, QB], F32, tag="pv", name=f"opv{b}")
                    # PV: the two batch chains interleave per j so neither
                    # waits for the other to fully drain.
                    if step >= PV_LAG:
                        j = step - PV_LAG
                        for b in range(B_LOC):
                            s = qb * SPQ + 2 * j + b
                            g, slot = group_of(s)
                            nc.tensor.matmul(
                                o_ps[b][:],
                                v16[:, b, j, :],
                                pt_tiles[g][:, slot * QB:(slot + 1) * QB],
                                start=(j == 0), stop=(j == NT - 1),
                                skip_group_check=True,
                            )
                for b in range(B_LOC):
                    # drain O'^T -> fp16 staging; transpose all 4 q-tiles into
                    # ONE psum bank (fp16 4*65=260 elems); single reciprocal of
                    # the 4 sums columns; one broadcast multiply; one store.
                    ot_sb = otp.tile([D + 1, QB], F16, tag="ot", name=f"ot{b}")
                    nc.vector.tensor_copy(ot_sb[:], o_ps[b][:])
                    tp4 = pvp.tile([P, QTPB, D + 2], F16, tag="pv", name=f"tp4_{b}")
                    for t in range(QTPB):
                        nc.tensor.transpose(
                            tp4[:, t, 0:D + 1], ot_sb[:, t * P:(t + 1) * P],
                            ident[:D + 1, :D + 1],
                        )
                    o_out = ostp.tile([P, QTPB, D], F32, tag="oo", name=f"oo{b}")
                    recip4 = ostp.tile([P, QTPB, 1], F32, tag="recip", name=f"recip{b}")
                    nc.vector.reciprocal(recip4[:], tp4[:, :, D:D + 1])
                    nc.vector.tensor_tensor(
                        o_out[:], tp4[:, :, 0:D],
                        recip4[:].to_broadcast((P, QTPB, D)),
                        mybir.AluOpType.mult,
                    )
                    o_dst = o_hbm[b, qs, :].rearrange("(t p) d -> p t d", p=P)
                    nc.sync.dma_start(o_dst, o_out[:])

    nc.compile()
    return nc


def get_nc():
    global _nc_cache
    if _nc_cache is None:
        _nc_cache = build()
    return _nc_cache


def kernel(queries: np.ndarray, keys: np.ndarray, values: np.ndarray) -> np.ndarray:
    from concourse.bass_utils import run_bass_kernel_spmd

    queries = np.ascontiguousarray(np.asarray(queries, dtype=np.float32))
    keys = np.ascontiguousarray(np.asarray(keys, dtype=np.float32))
    values = np.ascontiguousarray(np.asarray(values, dtype=np.float32))

    nc = get_nc()
    in_maps = []
    for c in range(N_CORES):
        sl = slice(c * B_LOC, (c + 1) * B_LOC)
        in_maps.append({
            "queries": queries[sl],
            "keys": keys[sl],
            "values": values[sl],
        })
    res = run_bass_kernel_spmd(nc, in_maps, core_ids=list(range(N_CORES)))
    return np.concatenate([r["out"] for r in res.results], axis=0)


if __name__ == "__main__":
    rng = np.random.default_rng(0)
    q = rng.standard_normal((B_FULL, N, D), dtype=np.float32)
    k = rng.standard_normal((B_FULL, N, D), dtype=np.float32)
    v = rng.standard_normal((B_FULL, N, D), dtype=np.float32)
    o = kernel(queries=q, keys=k, values=v)
    s = q @ k.transpose(0, 2, 1) / np.sqrt(D)
    w = np.exp(s - s.max(-1, keepdims=True))
    w /= w.sum(-1, keepdims=True)
    ref = w @ v
    err = np.abs(o - ref).max() / np.abs(ref).max()
    print("rel err:", err)



# revision 21
# speedup vs baseline: 1.1337x; 1.1337x over previous
"""Trainium2 Bass kernel: batched dense attention.

Full inputs: queries/keys/values [16, 2048, 64] fp32.
Shards batch dim across 8 NeuronCores (2 batches per core).

Per-core algorithm (batches A, B local):
  S^T[j, q] = K[j, :] . Q[q, :]           (PE, fp16 operands, fp32 PSUM)
  P^T = exp(S^T / 8)                       (ACT exp for most tiles; a tunable
                                            subset is offloaded to DVE+Pool
                                            via a phase-averaged Schraudolph
                                            bit-trick exp, see below)
  O[q, d'] = sum_j P^T[j, q] V'[j, d']     (PE; V' = [V | ones] so col 64 = sums)
  out[q, :] = O[q, 0:64] / O[q, 64]        (DVE reciprocal + broadcast mult)

Engine balance: exp for all 128 stream tiles on ACT alone costs ~64us while
PE needs only ~44us, so ~1/4 of the S^T groups bypass ACT:
  i0 = round(s*A + B0)  int16   (DVE tensor_scalar, fused convert, exact RNE)
  i1 = i0 - 512         int16   (Pool)
  t1 = bc16(i1)*sqrt2   fp16    (DVE, 4x mode)
  p  = t1 + bc16(i0)    fp16    (DVE, 2x mode)
which computes the average of two phase-shifted Schraudolph exp estimates
(elementwise |rel err| < 0.9%, end-to-end contribution ~5e-3, gate 2e-2).

Layout notes:
  - PV matmuls put q on the OUTPUT PARTITION dim (lhsT = P^T slice, rhs = V'),
    so each matmul's moving free dim is 65 instead of 512: PE time for the PV
    phase halves versus the O^T layout, and the output lands in the natural
    [q, d] layout (no final transposes, short drain tail).
  - Q^T / K^T built by PE transposes of natural tiles, batch-PAIRED so batch A
    lands on SBUF partitions 0-63 and batch B on 64-127.
  - S^T PSUM tiles grouped [2,3,3,...] per q-block so the first exp fires
    after just one j-tile of QK; groups are 3-wide elsewhere to amortize ACT
    per-instruction overhead.
  - fp32->fp16 input casts run on Pool (GPSIMD); DVE capacity is reserved for
    the exp offload; k/q/v loads spread across SP/DVE/ACT DMA queues so the
    head-of-kernel HWDGE serialization overlaps.
"""

import sys
for _p in ("/opt/trn_rl_repo", "/root/.axon_site/_ro/trn_rl_repo"):
    if _p not in sys.path:
        sys.path.insert(0, _p)

import numpy as np

import concourse.bass as bass
import concourse.mybir as mybir
from concourse import bacc
from concourse.tile import TileContext
from concourse.masks import make_identity

F32 = mybir.dt.float32
F16 = mybir.dt.float16
I16 = mybir.dt.int16
P = 128

N_CORES = 8
B_FULL, N, D = 16, 2048, 64
B_LOC = B_FULL // N_CORES          # 2 batches per core
NT = N // P                        # 16 tiles of 128 along q and j
TQ = NT // 2                       # 8 pair-interleaved staging tiles of 256
QB = 512                           # q-block (PSUM bank width in fp32)
NQB = N // QB                      # 4 q-blocks
QTPB = QB // P                     # 4 q-tiles per q-block
GROUP = 3                          # S^T stream tiles per exp instruction
SPQ = 2 * NT                       # 32 stream tiles per q-block
GPQ = 11                           # groups per q-block: [2,3,3,...,3]

# Schraudolph constants: exp(x*0.125) ~ avg of 2 phase-shifted estimates
EXP_A = 0.125 * 1024 * 1.4426950408889634          # 184.6649...
EXP_C = 56
EXP_B0 = float(15 * 1024 - EXP_C - 1024)           # nphase=2: fold the /2
SQRT2 = 1.4142135623730951

# which groups (local index within q-block) use the DVE/Pool offload path
OFF_LOCAL = {0: (5, 8), 1: (2, 5, 8), 2: (2, 5, 8), 3: (2, 5, 8)}
OFF_GROUPS = frozenset(qb * GPQ + g for qb, gs in OFF_LOCAL.items() for g in gs)

_nc_cache = None


def build():
    nc = bacc.Bacc(None, target_bir_lowering=False)
    q_hbm = nc.dram_tensor("queries", [B_LOC, N, D], F32, kind="ExternalInput")
    k_hbm = nc.dram_tensor("keys", [B_LOC, N, D], F32, kind="ExternalInput")
    v_hbm = nc.dram_tensor("values", [B_LOC, N, D], F32, kind="ExternalInput")
    o_hbm = nc.dram_tensor("out", [B_LOC, N, D], F32, kind="ExternalOutput")

    with TileContext(nc) as tc:
        with (
            tc.tile_pool(name="cst", bufs=1) as cst,
            tc.tile_pool(name="stage", bufs=2) as stage,
            tc.tile_pool(name="persist", bufs=1) as persist,
            tc.tile_pool(name="pt", bufs=14) as ptp,
            tc.tile_pool(name="off", bufs=2) as offp,
            tc.tile_pool(name="ost", bufs=4) as ostp,
            tc.tile_pool(name="st", bufs=2, space="PSUM") as stp,
            tc.tile_pool(name="pv", bufs=2, space="PSUM") as pvp,
        ):
            ident = cst.tile([P, P], F16)
            make_identity(nc, ident)

            # ---- persistent SBUF buffers ----
            # Q^T / K^T, batch-paired: rows 0-63 batch A (d), 64-127 batch B.
            qt = persist.tile([P, N], F16, tag="qt")
            kt = persist.tile([P, N], F16, tag="kt")
            # V' = [V | ones]: [128 j, b, t, e, 65] fp16 (pair-interleaved)
            v16 = persist.tile([P, B_LOC, TQ, 2, D + 1], F16, tag="v16")

            # Pair-interleaved staging: partition p of staged tile t holds the
            # TWO consecutive rows 256t+2p / 256t+2p+1 (e dim), so every DMA
            # descriptor is a 512B contiguous run (full bus efficiency; a 256B
            # run is charged 2x).  Row index within column-tile ct = 2t+e is a
            # fixed permutation shared by K and V (and by Q and the output
            # store), so attention math is unaffected.
            q_res = [q_hbm[b, :, :].rearrange("(t p e) d -> p t e d", p=P, e=2)
                     for b in range(B_LOC)]
            k_res = [k_hbm[b, :, :].rearrange("(t p e) d -> p t e d", p=P, e=2)
                     for b in range(B_LOC)]

            st32s, st16s = {}, {}
            for name in ("k", "q"):
                st32s[name] = stage.tile([P, TQ, B_LOC, 2, D], F32,
                                         tag=f"{name}s32", name=f"{name}s32")
                st16s[name] = stage.tile([P, TQ, B_LOC, 2, D], F16,
                                         tag=f"{name}s16", name=f"{name}s16")
            vs32 = stage.tile([P, B_LOC, TQ, 2, D], F32, tag="vs32")

            def load_chunk(name, t0, t1, eng, cast_eng=None):
                re_aps = k_res if name == "k" else q_res
                for b in range(B_LOC):
                    eng.dma_start(st32s[name][:, t0:t1, b],
                                  re_aps[b][:, t0:t1])
                    (cast_eng or nc.gpsimd).tensor_copy(
                        st16s[name][:, t0:t1, b], st32s[name][:, t0:t1, b])

            # Loads ordered by need; first k/q chunks on the ACT queue, the
            # rest on SP so the issue paths overlap (HWDGE itself is serial).
            # Head-critical chunks cast on DVE; everything later casts on
            # the otherwise-idle Pool engine so the in-order DVE queue never
            # holds a cast that waits on late DMA data (head-of-line).
            load_chunk("q", 0, 2, nc.sync, cast_eng=nc.vector)
            load_chunk("k", 0, 2, nc.scalar, cast_eng=nc.vector)
            load_chunk("k", 2, 5, nc.scalar, cast_eng=nc.vector)
            load_chunk("k", 5, TQ, nc.scalar, cast_eng=nc.vector)
            # V: cast into v16[..., :64] (per batch-half), ones col
            nc.gpsimd.memset(v16[:, :, :, :, D:D + 1], 1.0)
            v_res = [v_hbm[b, :, :].rearrange("(t p e) d -> p t e d", p=P, e=2)
                     for b in range(B_LOC)]
            for b in range(B_LOC):
                hs = slice(0, TQ // 2)
                nc.sync.dma_start(vs32[:, b, hs], v_res[b][:, hs])
                nc.gpsimd.tensor_copy(v16[:, b, hs, :, 0:D], vs32[:, b, hs])
            load_chunk("q", 2, 5, nc.sync)
            for b in range(B_LOC):
                hs = slice(TQ // 2, TQ)
                nc.sync.dma_start(vs32[:, b, hs], v_res[b][:, hs])
                nc.gpsimd.tensor_copy(v16[:, b, hs, :, 0:D], vs32[:, b, hs])
            load_chunk("q", 5, TQ, nc.sync)

            def prep_transpose(name, ct0, dst, pool=None, n=2):
                """PE-transpose column-tiles [ct0, ct0+n) of q/k into dst.

                2n [128,64]->[64,128] transposes (n column-tiles x 2 batches)
                fill the batch-paired rows of one PSUM tile; a single DVE
                copy drains all n*128 columns to SBUF."""
                pool = pool or pvp
                tp_ps = pool.tile([P, n, P], F16,
                                  tag="st" if pool is stp else "pv",
                                  name=f"tp_{name}{ct0}")
                for i in range(n):
                    t, e = (ct0 + i) // 2, (ct0 + i) % 2
                    for b in range(B_LOC):
                        nc.tensor.transpose(tp_ps[b * D:(b + 1) * D, i, :],
                                            st16s[name][:, t, b, e], ident[:])
                nc.vector.tensor_copy(dst[:, ct0 * P:(ct0 + n) * P],
                                      tp_ps[:])

            # K^T tiles j0-1 now; the rest are emitted lazily inside the qb0
            # loop (pool allocations are FIFO in emission order, so a transpose
            # emitted before its staging data lands would stall the slot ring).
            # ---- main loop ----
            # stream of S^T tiles: s = qb*32 + 2*j + b
            # groups per qb: [2, 3, 3, ..., 3] (first exp after one j-tile)
            st_tiles = {}      # group -> psum tile
            pt_tiles = {}      # group -> sbuf fp16 tile

            def group_of(s):
                qb_, sl = divmod(s, SPQ)
                if qb_ == NQB - 1:
                    return qb_ * GPQ + min(sl // 3, GPQ - 1), \
                        sl % 3 if sl < 30 else sl - 30
                if sl < 2:
                    return qb_ * GPQ, sl
                return qb_ * GPQ + 1 + (sl - 2) // GROUP, (sl - 2) % GROUP

            def tiles_in_group(g):
                if g // GPQ == NQB - 1:
                    return 2 if g % GPQ == GPQ - 1 else GROUP
                return 2 if g % GPQ == 0 else GROUP

            off_i0 = {}

            def maybe_exp(g):
                """emit exp for group g once all its stream tiles are written.

                ACT path is one instruction.  The offload path emits only the
                PSUM-reading TS here (frees the S^T slot ring); the remaining
                three cheap SBUF ops are emitted ~2 j-steps later via
                finish_off so the in-order DVE queue interleaves other work
                and the chain latency hides under the PV lag."""
                w = tiles_in_group(g) * QB
                if g in OFF_GROUPS:
                    i0 = offp.tile([P, GROUP * QB], I16, tag="i0",
                                   name=f"i0g{g}")
                    nc.vector.tensor_scalar(i0[:, :w], st_tiles[g][:, :w],
                                            EXP_A, EXP_B0,
                                            mybir.AluOpType.mult,
                                            mybir.AluOpType.add)
                    off_i0[g] = i0
                    return
                pt_t = ptp.tile([P, GROUP * QB], F16, tag="pt", name=f"ptg{g}")
                nc.scalar.activation(
                    pt_t[:, :w], st_tiles[g][:, :w],
                    mybir.ActivationFunctionType.Exp, scale=0.125,
                )
                pt_tiles[g] = pt_t

            def finish_off(g):
                w = tiles_in_group(g) * QB
                i0 = off_i0.pop(g)
                pt_t = ptp.tile([P, GROUP * QB], F16, tag="pt", name=f"ptg{g}")
                i1 = offp.tile([P, GROUP * QB], I16, tag="i1", name=f"i1g{g}")
                nc.vector.tensor_scalar(i1[:, :w], i0[:, :w], 512, None,
                                        mybir.AluOpType.subtract)
                t1 = offp.tile([P, GROUP * QB], F16, tag="t1", name=f"t1g{g}")
                nc.vector.tensor_scalar(t1[:, :w], i1[:, :w].bitcast(F16),
                                        SQRT2, None, mybir.AluOpType.mult)
                nc.vector.tensor_tensor(pt_t[:, :w], t1[:, :w],
                                        i0[:, :w].bitcast(F16),
                                        mybir.AluOpType.add)
                pt_tiles[g] = pt_t

            def pt_slice(s, t4):
                """lhsT slice [128 j, 128 q] for stream tile s, q-subtile t4"""
                g, slot = group_of(s)
                c0 = slot * QB + t4 * P
                return pt_tiles[g][:, c0:c0 + P]

            # Q^T tiles for q-block 0
            prep_transpose("q", 0, qt, pool=pvp)
            prep_transpose("q", 2, qt, pool=stp)
            prep_transpose("k", 0, kt, pool=pvp)

            # ---- per-q-block loop ----
            o_acc = {}
            for qb in range(NQB):
                qs = slice(qb * QB, (qb + 1) * QB)
                PV_LAG = 14 if qb == 0 else 7
                pending_off = {}
                for step in range(NT + PV_LAG):
                    if step < NT:
                        j = step
                        if qb == 0 and j % 4 == 0 and j + 2 < NT:
                            prep_transpose("k", j + 2, kt,
                                           n=(4 if j + 6 <= NT else 2))
                        if qb + 1 < NQB and 3 <= j < 5:
                            prep_transpose(
                                "q", (qb + 1) * QTPB + 2 * (j - 3), qt)
                        for b in range(B_LOC):
                            s = qb * SPQ + 2 * j + b
                            g, slot = group_of(s)
                            if slot == 0:
                                st_tiles[g] = stp.tile([P, GROUP * QB], F32,
                                                       tag="st", name=f"stg{g}")
                            rows = slice(b * D, (b + 1) * D)
                            nc.tensor.matmul(
                                st_tiles[g][:, slot * QB:(slot + 1) * QB],
                                kt[rows, j * P:(j + 1) * P],
                                qt[rows, qs],
                                start=True, stop=True,
                            )
                            if slot == tiles_in_group(g) - 1:
                                maybe_exp(g)
                                if g in OFF_GROUPS:
                                    pending_off.setdefault(step + 2,
                                                           []).append(g)
                    for g in pending_off.pop(step, ()):
                        finish_off(g)
                    if step == PV_LAG:
                        for b in range(B_LOC):
                            o_acc[b] = pvp.tile([P, QTPB, D + 1], F32, tag="pv",
                                                name=f"oacc{qb}_{b}")
                    if step >= PV_LAG and step - PV_LAG < NT:
                        j = step - PV_LAG
                        for b in range(B_LOC):
                            s = qb * SPQ + 2 * j + b
                            for t4 in range(QTPB):
                                # start=True zeroes the ENTIRE psum bank, so
                                # only the first matmul into this o_acc bank
                                # may set it; later regions accumulate onto
                                # the already-zeroed bank.
                                nc.tensor.matmul(
                                    o_acc[b][:, t4, :],
                                    pt_slice(s, t4),
                                    v16[:, b, j // 2, j % 2, :],
                                    start=(j == 0 and t4 == 0),
                                    stop=(j == NT - 1),
                                    skip_group_check=True,
                                )
                for b in range(B_LOC):
                    recip4 = ostp.tile([P, QTPB, 1], F32, tag="recip",
                                       name=f"recip{qb}_{b}")
                    nc.vector.reciprocal(recip4[:], o_acc[b][:, :, D:D + 1])
                    o_out = ostp.tile([P, QTPB, D], F32, tag="oo",
                                      name=f"oo{qb}_{b}")
                    nc.vector.tensor_tensor(
                        o_out[:], o_acc[b][:, :, 0:D],
                        recip4[:].to_broadcast((P, QTPB, D)),
                        mybir.AluOpType.mult,
                    )
                    o_dst = o_hbm[b, :, :].rearrange(
                        "(t p e) d -> p t e d", p=P,
                        e=2)[:, 2 * qb:2 * qb + 2]
                    nc.sync.dma_start(o_dst, o_out[:])

    nc.compile()
    return nc


def get_nc():
    global _nc_cache
    if _nc_cache is None:
        _nc_cache = build()
    return _nc_cache


def kernel(queries: np.ndarray, keys: np.ndarray, values: np.ndarray) -> np.ndarray:
    from concourse.bass_utils import run_bass_kernel_spmd

    queries = np.ascontiguousarray(np.asarray(queries, dtype=np.float32))
    keys = np.ascontiguousarray(np.asarray(keys, dtype=np.float32))
    values = np.ascontiguousarray(np.asarray(values, dtype=np.float32))

    nc = get_nc()
    in_maps = []
    for c in range(N_CORES):
        sl = slice(c * B_LOC, (c + 1) * B_LOC)
        in_maps.append({
            "queries": queries[sl],
            "keys": keys[sl],
            "values": values[sl],
        })
    res = run_bass_kernel_spmd(nc, in_maps, core_ids=list(range(N_CORES)))
    return np.concatenate([r["out"] for r in res.results], axis=0)


if __name__ == "__main__":
    rng = np.random.default_rng(0)
    q = rng.standard_normal((B_FULL, N, D), dtype=np.float32)
    k = rng.standard_normal((B_FULL, N, D), dtype=np.float32)
    v = rng.standard_normal((B_FULL, N, D), dtype=np.float32)
    o = kernel(queries=q, keys=k, values=v)
    s = q @ k.transpose(0, 2, 1) / np.sqrt(D)
    w = np.exp(s - s.max(-1, keepdims=True))
    w /= w.sum(-1, keepdims=True)
    ref = w @ v
    err = np.abs(o - ref).max() / np.abs(ref).max()
    print("rel err:", err)


# revision 22
# speedup vs baseline: 1.1358x; 1.0018x over previous
"""Trainium2 Bass kernel: batched dense attention.

Full inputs: queries/keys/values [16, 2048, 64] fp32.
Shards batch dim across 8 NeuronCores (2 batches per core).

Per-core algorithm (batches A, B local):
  S^T[j, q] = K[j, :] . Q[q, :]           (PE, fp16 operands, fp32 PSUM)
  P^T = exp(S^T / 8)                       (ACT exp for most tiles; a tunable
                                            subset is offloaded to DVE+Pool
                                            via a phase-averaged Schraudolph
                                            bit-trick exp, see below)
  O[q, d'] = sum_j P^T[j, q] V'[j, d']     (PE; V' = [V | ones] so col 64 = sums)
  out[q, :] = O[q, 0:64] / O[q, 64]        (DVE reciprocal + broadcast mult)

Engine balance: exp for all 128 stream tiles on ACT alone costs ~64us while
PE needs only ~46us, so 11 of the 44 S^T groups bypass ACT via a DVE chain:
  i0 = round(s*A + B0)  int16   (tensor_scalar, fused convert, exact RNE)
  i1 = i0 - 512         int16   (tensor_scalar, 4x mode)
  t1 = bc16(i1)*sqrt2   fp16    (tensor_scalar, 4x mode)
  p  = t1 + bc16(i0)    fp16    (tensor_tensor, 2x mode)
which computes the average of two phase-shifted Schraudolph exp estimates
(elementwise |rel err| < 0.9%, end-to-end ~2.5e-3 vs the 2e-2 gate).  The
PSUM-reading i0 step is emitted at group completion (frees the S^T ring);
the three cheap SBUF ops are deferred 2 j-steps so the in-order DVE queue
interleaves other work and the chain latency hides under the PV lag.
Engine busy: ACT ~48us, DVE ~47us, PE ~46us, Pool ~12us.

PSUM note: matmul start=True zeroes the ENTIRE psum bank, so the four
interleaved per-t4 accumulation regions of an o_acc bank set start only on
the very first matmul into the bank (see the PV loop).

Layout notes:
  - PV matmuls put q on the OUTPUT PARTITION dim (lhsT = P^T slice, rhs = V'),
    so each matmul's moving free dim is 65 instead of 512: PE time for the PV
    phase halves versus the O^T layout, and the output lands in the natural
    [q, d] layout (no final transposes, short drain tail).
  - Q^T / K^T built by PE transposes of natural tiles, batch-PAIRED so batch A
    lands on SBUF partitions 0-63 and batch B on 64-127.
  - S^T PSUM tiles grouped [2,3,3,...] per q-block so the first exp fires
    after just one j-tile of QK; groups are 3-wide elsewhere to amortize ACT
    per-instruction overhead.
  - Head-critical input casts (first k/q chunks) run on DVE; later casts on
    Pool (GPSIMD) so the in-order DVE queue never holds a cast waiting on
    late DMA data.  k/q/v loads are chunked by first-use and spread across
    the SP and ACT DMA queues to overlap HWDGE issue serialization.
"""

import sys
for _p in ("/opt/trn_rl_repo", "/root/.axon_site/_ro/trn_rl_repo"):
    if _p not in sys.path:
        sys.path.insert(0, _p)

import numpy as np

import concourse.bass as bass
import concourse.mybir as mybir
from concourse import bacc
from concourse.tile import TileContext
from concourse.masks import make_identity

F32 = mybir.dt.float32
F16 = mybir.dt.float16
I16 = mybir.dt.int16
P = 128

N_CORES = 8
B_FULL, N, D = 16, 2048, 64
B_LOC = B_FULL // N_CORES          # 2 batches per core
NT = N // P                        # 16 tiles of 128 along q and j
TQ = NT // 2                       # 8 pair-interleaved staging tiles of 256
QB = 512                           # q-block (PSUM bank width in fp32)
NQB = N // QB                      # 4 q-blocks
QTPB = QB // P                     # 4 q-tiles per q-block
GROUP = 3                          # S^T stream tiles per exp instruction
SPQ = 2 * NT                       # 32 stream tiles per q-block
GPQ = 11                           # groups per q-block: [2,3,3,...,3]

# Schraudolph constants: exp(x*0.125) ~ avg of 2 phase-shifted estimates
EXP_A = 0.125 * 1024 * 1.4426950408889634          # 184.6649...
EXP_C = 56
EXP_B0 = float(15 * 1024 - EXP_C - 1024)           # nphase=2: fold the /2
SQRT2 = 1.4142135623730951

# which groups (local index within q-block) use the DVE/Pool offload path
OFF_LOCAL = {0: (5, 8), 1: (2, 5, 8), 2: (2, 5, 8), 3: (2, 5, 7)}
OFF_GROUPS = frozenset(qb * GPQ + g for qb, gs in OFF_LOCAL.items() for g in gs)

_nc_cache = None


def build():
    nc = bacc.Bacc(None, target_bir_lowering=False)
    q_hbm = nc.dram_tensor("queries", [B_LOC, N, D], F32, kind="ExternalInput")
    k_hbm = nc.dram_tensor("keys", [B_LOC, N, D], F32, kind="ExternalInput")
    v_hbm = nc.dram_tensor("values", [B_LOC, N, D], F32, kind="ExternalInput")
    o_hbm = nc.dram_tensor("out", [B_LOC, N, D], F32, kind="ExternalOutput")

    with TileContext(nc) as tc:
        with (
            tc.tile_pool(name="cst", bufs=1) as cst,
            tc.tile_pool(name="stage", bufs=2) as stage,
            tc.tile_pool(name="persist", bufs=1) as persist,
            tc.tile_pool(name="pt", bufs=14) as ptp,
            tc.tile_pool(name="off", bufs=2) as offp,
            tc.tile_pool(name="ost", bufs=4) as ostp,
            tc.tile_pool(name="st", bufs=2, space="PSUM") as stp,
            tc.tile_pool(name="pv", bufs=2, space="PSUM") as pvp,
        ):
            ident = cst.tile([P, P], F16)
            make_identity(nc, ident)

            # ---- persistent SBUF buffers ----
            # Q^T / K^T, batch-paired: rows 0-63 batch A (d), 64-127 batch B.
            qt = persist.tile([P, N], F16, tag="qt")
            kt = persist.tile([P, N], F16, tag="kt")
            # V' = [V | ones]: [128 j, b, t, e, 65] fp16 (pair-interleaved)
            v16 = persist.tile([P, B_LOC, TQ, 2, D + 1], F16, tag="v16")

            # Pair-interleaved staging: partition p of staged tile t holds the
            # TWO consecutive rows 256t+2p / 256t+2p+1 (e dim), so every DMA
            # descriptor is a 512B contiguous run (full bus efficiency; a 256B
            # run is charged 2x).  Row index within column-tile ct = 2t+e is a
            # fixed permutation shared by K and V (and by Q and the output
            # store), so attention math is unaffected.
            q_res = [q_hbm[b, :, :].rearrange("(t p e) d -> p t e d", p=P, e=2)
                     for b in range(B_LOC)]
            k_res = [k_hbm[b, :, :].rearrange("(t p e) d -> p t e d", p=P, e=2)
                     for b in range(B_LOC)]

            st32s, st16s = {}, {}
            for name in ("k", "q"):
                st32s[name] = stage.tile([P, TQ, B_LOC, 2, D], F32,
                                         tag=f"{name}s32", name=f"{name}s32")
                st16s[name] = stage.tile([P, TQ, B_LOC, 2, D], F16,
                                         tag=f"{name}s16", name=f"{name}s16")
            vs32 = stage.tile([P, B_LOC, TQ, 2, D], F32, tag="vs32")

            def load_chunk(name, t0, t1, eng, cast_eng=None):
                re_aps = k_res if name == "k" else q_res
                for b in range(B_LOC):
                    eng.dma_start(st32s[name][:, t0:t1, b],
                                  re_aps[b][:, t0:t1])
                    (cast_eng or nc.gpsimd).tensor_copy(
                        st16s[name][:, t0:t1, b], st32s[name][:, t0:t1, b])

            # Loads ordered by need; first k/q chunks on the ACT queue, the
            # rest on SP so the issue paths overlap (HWDGE itself is serial).
            # Head-critical chunks cast on DVE; everything later casts on
            # the otherwise-idle Pool engine so the in-order DVE queue never
            # holds a cast that waits on late DMA data (head-of-line).
            load_chunk("q", 0, 2, nc.sync, cast_eng=nc.vector)
            load_chunk("k", 0, 2, nc.scalar, cast_eng=nc.vector)
            load_chunk("k", 2, 5, nc.scalar, cast_eng=nc.vector)
            load_chunk("k", 5, TQ, nc.scalar, cast_eng=nc.vector)
            # V: cast into v16[..., :64] (per batch-half), ones col
            nc.gpsimd.memset(v16[:, :, :, :, D:D + 1], 1.0)
            v_res = [v_hbm[b, :, :].rearrange("(t p e) d -> p t e d", p=P, e=2)
                     for b in range(B_LOC)]
            for b in range(B_LOC):
                hs = slice(0, TQ // 2)
                nc.sync.dma_start(vs32[:, b, hs], v_res[b][:, hs])
                nc.gpsimd.tensor_copy(v16[:, b, hs, :, 0:D], vs32[:, b, hs])
            load_chunk("q", 2, 5, nc.sync)
            for b in range(B_LOC):
                hs = slice(TQ // 2, TQ)
                nc.sync.dma_start(vs32[:, b, hs], v_res[b][:, hs])
                nc.gpsimd.tensor_copy(v16[:, b, hs, :, 0:D], vs32[:, b, hs])
            load_chunk("q", 5, TQ, nc.sync)

            def prep_transpose(name, ct0, dst, pool=None, n=2):
                """PE-transpose column-tiles [ct0, ct0+n) of q/k into dst.

                2n [128,64]->[64,128] transposes (n column-tiles x 2 batches)
                fill the batch-paired rows of one PSUM tile; a single DVE
                copy drains all n*128 columns to SBUF."""
                pool = pool or pvp
                tp_ps = pool.tile([P, n, P], F16,
                                  tag="st" if pool is stp else "pv",
                                  name=f"tp_{name}{ct0}")
                for i in range(n):
                    t, e = (ct0 + i) // 2, (ct0 + i) % 2
                    for b in range(B_LOC):
                        nc.tensor.transpose(tp_ps[b * D:(b + 1) * D, i, :],
                                            st16s[name][:, t, b, e], ident[:])
                nc.vector.tensor_copy(dst[:, ct0 * P:(ct0 + n) * P],
                                      tp_ps[:])

            # K^T tiles j0-1 now; the rest are emitted lazily inside the qb0
            # loop (pool allocations are FIFO in emission order, so a transpose
            # emitted before its staging data lands would stall the slot ring).
            # ---- main loop ----
            # stream of S^T tiles: s = qb*32 + 2*j + b
            # groups per qb: [2, 3, 3, ..., 3] (first exp after one j-tile)
            st_tiles = {}      # group -> psum tile
            pt_tiles = {}      # group -> sbuf fp16 tile

            def group_of(s):
                qb_, sl = divmod(s, SPQ)
                if qb_ == NQB - 1:
                    return qb_ * GPQ + min(sl // 3, GPQ - 1), \
                        sl % 3 if sl < 30 else sl - 30
                if sl < 2:
                    return qb_ * GPQ, sl
                return qb_ * GPQ + 1 + (sl - 2) // GROUP, (sl - 2) % GROUP

            def tiles_in_group(g):
                if g // GPQ == NQB - 1:
                    return 2 if g % GPQ == GPQ - 1 else GROUP
                return 2 if g % GPQ == 0 else GROUP

            off_i0 = {}

            def maybe_exp(g):
                """emit exp for group g once all its stream tiles are written.

                ACT path is one instruction.  The offload path emits only the
                PSUM-reading TS here (frees the S^T slot ring); the remaining
                three cheap SBUF ops are emitted ~2 j-steps later via
                finish_off so the in-order DVE queue interleaves other work
                and the chain latency hides under the PV lag."""
                w = tiles_in_group(g) * QB
                if g in OFF_GROUPS:
                    i0 = offp.tile([P, GROUP * QB], I16, tag="i0",
                                   name=f"i0g{g}")
                    nc.vector.tensor_scalar(i0[:, :w], st_tiles[g][:, :w],
                                            EXP_A, EXP_B0,
                                            mybir.AluOpType.mult,
                                            mybir.AluOpType.add)
                    off_i0[g] = i0
                    return
                pt_t = ptp.tile([P, GROUP * QB], F16, tag="pt", name=f"ptg{g}")
                nc.scalar.activation(
                    pt_t[:, :w], st_tiles[g][:, :w],
                    mybir.ActivationFunctionType.Exp, scale=0.125,
                )
                pt_tiles[g] = pt_t

            def finish_off(g):
                w = tiles_in_group(g) * QB
                i0 = off_i0.pop(g)
                pt_t = ptp.tile([P, GROUP * QB], F16, tag="pt", name=f"ptg{g}")
                i1 = offp.tile([P, GROUP * QB], I16, tag="i1", name=f"i1g{g}")
                nc.vector.tensor_scalar(i1[:, :w], i0[:, :w], 512, None,
                                        mybir.AluOpType.subtract)
                t1 = offp.tile([P, GROUP * QB], F16, tag="t1", name=f"t1g{g}")
                nc.vector.tensor_scalar(t1[:, :w], i1[:, :w].bitcast(F16),
                                        SQRT2, None, mybir.AluOpType.mult)
                nc.vector.tensor_tensor(pt_t[:, :w], t1[:, :w],
                                        i0[:, :w].bitcast(F16),
                                        mybir.AluOpType.add)
                pt_tiles[g] = pt_t

            def pt_slice(s, t4):
                """lhsT slice [128 j, 128 q] for stream tile s, q-subtile t4"""
                g, slot = group_of(s)
                c0 = slot * QB + t4 * P
                return pt_tiles[g][:, c0:c0 + P]

            # Q^T tiles for q-block 0
            prep_transpose("q", 0, qt, pool=pvp)
            prep_transpose("q", 2, qt, pool=stp)
            prep_transpose("k", 0, kt, pool=pvp)

            # ---- per-q-block loop ----
            o_acc = {}
            for qb in range(NQB):
                qs = slice(qb * QB, (qb + 1) * QB)
                PV_LAG = 14 if qb == 0 else 7
                pending_off = {}
                for step in range(NT + PV_LAG):
                    if step < NT:
                        j = step
                        if qb == 0 and j % 4 == 0 and j + 2 < NT:
                            prep_transpose("k", j + 2, kt,
                                           n=(4 if j + 6 <= NT else 2))
                        if qb + 1 < NQB and 3 <= j < 5:
                            prep_transpose(
                                "q", (qb + 1) * QTPB + 2 * (j - 3), qt)
                        for b in range(B_LOC):
                            s = qb * SPQ + 2 * j + b
                            g, slot = group_of(s)
                            if slot == 0:
                                st_tiles[g] = stp.tile([P, GROUP * QB], F32,
                                                       tag="st", name=f"stg{g}")
                            rows = slice(b * D, (b + 1) * D)
                            nc.tensor.matmul(
                                st_tiles[g][:, slot * QB:(slot + 1) * QB],
                                kt[rows, j * P:(j + 1) * P],
                                qt[rows, qs],
                                start=True, stop=True,
                            )
                            if slot == tiles_in_group(g) - 1:
                                maybe_exp(g)
                                if g in OFF_GROUPS:
                                    pending_off.setdefault(step + 2,
                                                           []).append(g)
                    for g in pending_off.pop(step, ()):
                        finish_off(g)
                    if step == PV_LAG:
                        for b in range(B_LOC):
                            o_acc[b] = pvp.tile([P, QTPB, D + 1], F32, tag="pv",
                                                name=f"oacc{qb}_{b}")
                    if step >= PV_LAG and step - PV_LAG < NT:
                        j = step - PV_LAG
                        for b in range(B_LOC):
                            s = qb * SPQ + 2 * j + b
                            for t4 in range(QTPB):
                                # start=True zeroes the ENTIRE psum bank, so
                                # only the first matmul into this o_acc bank
                                # may set it; later regions accumulate onto
                                # the already-zeroed bank.
                                nc.tensor.matmul(
                                    o_acc[b][:, t4, :],
                                    pt_slice(s, t4),
                                    v16[:, b, j // 2, j % 2, :],
                                    start=(j == 0 and t4 == 0),
                                    stop=(j == NT - 1),
                                    skip_group_check=True,
                                )
                for b in range(B_LOC):
                    recip4 = ostp.tile([P, QTPB, 1], F32, tag="recip",
                                       name=f"recip{qb}_{b}")
                    nc.vector.reciprocal(recip4[:], o_acc[b][:, :, D:D + 1])
                    o_out = ostp.tile([P, QTPB, D], F32, tag="oo",
                                      name=f"oo{qb}_{b}")
                    nc.vector.tensor_tensor(
                        o_out[:], o_acc[b][:, :, 0:D],
                        recip4[:].to_broadcast((P, QTPB, D)),
                        mybir.AluOpType.mult,
                    )
                    o_dst = o_hbm[b, :, :].rearrange(
                        "(t p e) d -> p t e d", p=P,
                        e=2)[:, 2 * qb:2 * qb + 2]
                    nc.sync.dma_start(o_dst, o_out[:])

    nc.compile()
    return nc


def get_nc():
    global _nc_cache
    if _nc_cache is None:
        _nc_cache = build()
    return _nc_cache


def kernel(queries: np.ndarray, keys: np.ndarray, values: np.ndarray) -> np.ndarray:
    from concourse.bass_utils import run_bass_kernel_spmd

    queries = np.ascontiguousarray(np.asarray(queries, dtype=np.float32))
    keys = np.ascontiguousarray(np.asarray(keys, dtype=np.float32))
    values = np.ascontiguousarray(np.asarray(values, dtype=np.float32))

    nc = get_nc()
    in_maps = []
    for c in range(N_CORES):
        sl = slice(c * B_LOC, (c + 1) * B_LOC)
        in_maps.append({
            "queries": queries[sl],
            "keys": keys[sl],
            "values": values[sl],
        })
    res = run_bass_kernel_spmd(nc, in_maps, core_ids=list(range(N_CORES)))
    return np.concatenate([r["out"] for r in res.results], axis=0)


if __name__ == "__main__":
    rng = np.random.default_rng(0)
    q = rng.standard_normal((B_FULL, N, D), dtype=np.float32)
    k = rng.standard_normal((B_FULL, N, D), dtype=np.float32)
    v = rng.standard_normal((B_FULL, N, D), dtype=np.float32)
    o = kernel(queries=q, keys=k, values=v)
    s = q @ k.transpose(0, 2, 1) / np.sqrt(D)
    w = np.exp(s - s.max(-1, keepdims=True))
    w /= w.sum(-1, keepdims=True)
    ref = w @ v
    err = np.abs(o - ref).max() / np.abs(ref).max()
    print("rel err:", err)


# revision 23
# speedup vs baseline: 1.1429x; 1.0062x over previous
"""Trainium2 Bass kernel: batched dense attention.

Full inputs: queries/keys/values [16, 2048, 64] fp32.
Shards batch dim across 8 NeuronCores (2 batches per core).

Per-core algorithm (batches A, B local):
  S^T[j, q] = K[j, :] . Q[q, :]           (PE, fp16 operands, fp32 PSUM)
  P^T = exp(S^T / 8)                       (ACT exp for most tiles; a tunable
                                            subset is offloaded to DVE+Pool
                                            via a phase-averaged Schraudolph
                                            bit-trick exp, see below)
  O[q, d'] = sum_j P^T[j, q] V'[j, d']     (PE; V' = [V | ones] so col 64 = sums)
  out[q, :] = O[q, 0:64] / O[q, 64]        (DVE reciprocal + broadcast mult)

Engine balance: exp for all 128 stream tiles on ACT alone costs ~64us while
PE needs only ~46us, so 11 of the 44 S^T groups bypass ACT via a DVE chain:
  i0 = round(s*A + B0)  int16   (tensor_scalar, fused convert, exact RNE)
  i1 = i0 - 512         int16   (tensor_scalar, 4x mode)
  t1 = bc16(i1)*sqrt2   fp16    (tensor_scalar, 4x mode)
  p  = t1 + bc16(i0)    fp16    (tensor_tensor, 2x mode)
which computes the average of two phase-shifted Schraudolph exp estimates
(elementwise |rel err| < 0.9%, end-to-end ~2.5e-3 vs the 2e-2 gate).  The
PSUM-reading i0 step is emitted at group completion (frees the S^T ring);
the three cheap SBUF ops are deferred 2 j-steps so the in-order DVE queue
interleaves other work and the chain latency hides under the PV lag.
Engine busy: ACT ~48us, DVE ~47us, PE ~46us, Pool ~12us.

PSUM note: matmul start=True zeroes the ENTIRE psum bank, so the four
interleaved per-t4 accumulation regions of an o_acc bank set start only on
the very first matmul into the bank (see the PV loop).

Layout notes:
  - PV matmuls put q on the OUTPUT PARTITION dim (lhsT = P^T slice, rhs = V'),
    so each matmul's moving free dim is 65 instead of 512: PE time for the PV
    phase halves versus the O^T layout, and the output lands in the natural
    [q, d] layout (no final transposes, short drain tail).
  - Q^T / K^T built by PE transposes of natural tiles, batch-PAIRED so batch A
    lands on SBUF partitions 0-63 and batch B on 64-127.
  - S^T PSUM tiles grouped [2,3,3,...] per q-block so the first exp fires
    after just one j-tile of QK; groups are 3-wide elsewhere to amortize ACT
    per-instruction overhead.
  - Head-critical input casts (first k/q chunks) run on DVE; later casts on
    Pool (GPSIMD) so the in-order DVE queue never holds a cast waiting on
    late DMA data.  k/q/v loads are chunked by first-use and spread across
    the SP and ACT DMA queues to overlap HWDGE issue serialization.
"""

import sys
for _p in ("/opt/trn_rl_repo", "/root/.axon_site/_ro/trn_rl_repo"):
    if _p not in sys.path:
        sys.path.insert(0, _p)

import numpy as np

import concourse.bass as bass
import concourse.mybir as mybir
from concourse import bacc
from concourse.tile import TileContext
from concourse.masks import make_identity

F32 = mybir.dt.float32
F16 = mybir.dt.float16
I16 = mybir.dt.int16
P = 128

N_CORES = 8
B_FULL, N, D = 16, 2048, 64
B_LOC = B_FULL // N_CORES          # 2 batches per core
NT = N // P                        # 16 tiles of 128 along q and j
TQ = NT // 2                       # 8 pair-interleaved staging tiles of 256
QB = 512                           # q-block (PSUM bank width in fp32)
NQB = N // QB                      # 4 q-blocks
QTPB = QB // P                     # 4 q-tiles per q-block
GROUP = 3                          # S^T stream tiles per exp instruction
SPQ = 2 * NT                       # 32 stream tiles per q-block
GPQ = 11                           # groups per q-block: [2,3,3,...,3]

# Schraudolph constants: exp(x*0.125) ~ avg of 2 phase-shifted estimates
EXP_A = 0.125 * 1024 * 1.4426950408889634          # 184.6649...
EXP_C = 56
EXP_B0 = float(15 * 1024 - EXP_C - 1024)           # nphase=2: fold the /2
SQRT2 = 1.4142135623730951

# which groups (local index within q-block) use the DVE/Pool offload path
OFF_LOCAL = {0: (5, 8), 1: (2, 5, 8), 2: (2, 5, 8), 3: (2, 5, 7)}
OFF_GROUPS = frozenset(qb * GPQ + g for qb, gs in OFF_LOCAL.items() for g in gs)

_nc_cache = None


def build():
    nc = bacc.Bacc(None, target_bir_lowering=False)
    q_hbm = nc.dram_tensor("queries", [B_LOC, N, D], F32, kind="ExternalInput")
    k_hbm = nc.dram_tensor("keys", [B_LOC, N, D], F32, kind="ExternalInput")
    v_hbm = nc.dram_tensor("values", [B_LOC, N, D], F32, kind="ExternalInput")
    o_hbm = nc.dram_tensor("out", [B_LOC, N, D], F32, kind="ExternalOutput")

    with TileContext(nc) as tc:
        with (
            tc.tile_pool(name="cst", bufs=1) as cst,
            tc.tile_pool(name="stage", bufs=2) as stage,
            tc.tile_pool(name="persist", bufs=1) as persist,
            tc.tile_pool(name="pt", bufs=14) as ptp,
            tc.tile_pool(name="off", bufs=2) as offp,
            tc.tile_pool(name="ost", bufs=4) as ostp,
            tc.tile_pool(name="st", bufs=2, space="PSUM") as stp,
            tc.tile_pool(name="pv", bufs=2, space="PSUM") as pvp,
        ):
            ident = cst.tile([P, P], F16)
            make_identity(nc, ident)

            # ---- persistent SBUF buffers ----
            # Q^T / K^T, batch-paired: rows 0-63 batch A (d), 64-127 batch B.
            qt = persist.tile([P, N], F16, tag="qt")
            kt = persist.tile([P, N], F16, tag="kt")
            # V' = [V | ones]: [128 j, b, t, e, 65] fp16 (pair-interleaved)
            v16 = persist.tile([P, B_LOC, TQ, 2, D + 1], F16, tag="v16")

            # Pair-interleaved staging: partition p of staged tile t holds the
            # TWO consecutive rows 256t+2p / 256t+2p+1 (e dim), so every DMA
            # descriptor is a 512B contiguous run (full bus efficiency; a 256B
            # run is charged 2x).  Row index within column-tile ct = 2t+e is a
            # fixed permutation shared by K and V (and by Q and the output
            # store), so attention math is unaffected.
            q_res = [q_hbm[b, :, :].rearrange("(t p e) d -> p t e d", p=P, e=2)
                     for b in range(B_LOC)]
            k_res = [k_hbm[b, :, :].rearrange("(t p e) d -> p t e d", p=P, e=2)
                     for b in range(B_LOC)]

            st32s, st16s = {}, {}
            for name in ("k", "q"):
                st32s[name] = stage.tile([P, TQ, B_LOC, 2, D], F32,
                                         tag=f"{name}s32", name=f"{name}s32")
                # transpose-friendly layout: for fixed (t, e) the (b, d)
                # block is one contiguous 128-wide free dim
                st16s[name] = stage.tile([P, TQ, 2, B_LOC, D], F16,
                                         tag=f"{name}s16", name=f"{name}s16")
            vs32 = stage.tile([P, B_LOC, TQ, 2, D], F32, tag="vs32")

            def load_chunk(name, t0, t1, eng, cast_eng=None):
                re_aps = k_res if name == "k" else q_res
                for b in range(B_LOC):
                    eng.dma_start(st32s[name][:, t0:t1, b],
                                  re_aps[b][:, t0:t1])
                    (cast_eng or nc.gpsimd).tensor_copy(
                        st16s[name][:, t0:t1, :, b, :],
                        st32s[name][:, t0:t1, b])

            # Loads ordered by need; first k/q chunks on the ACT queue, the
            # rest on SP so the issue paths overlap (HWDGE itself is serial).
            # Head-critical chunks cast on DVE; everything later casts on
            # the otherwise-idle Pool engine so the in-order DVE queue never
            # holds a cast that waits on late DMA data (head-of-line).
            load_chunk("q", 0, 2, nc.sync, cast_eng=nc.vector)
            load_chunk("k", 0, 2, nc.scalar, cast_eng=nc.vector)
            load_chunk("k", 2, 5, nc.scalar, cast_eng=nc.vector)
            load_chunk("k", 5, TQ, nc.scalar, cast_eng=nc.vector)
            # V: cast into v16[..., :64] (per batch-half), ones col
            nc.gpsimd.memset(v16[:, :, :, :, D:D + 1], 1.0)
            v_res = [v_hbm[b, :, :].rearrange("(t p e) d -> p t e d", p=P, e=2)
                     for b in range(B_LOC)]
            for b in range(B_LOC):
                hs = slice(0, TQ // 2)
                nc.sync.dma_start(vs32[:, b, hs], v_res[b][:, hs])
                nc.gpsimd.tensor_copy(v16[:, b, hs, :, 0:D], vs32[:, b, hs])
            load_chunk("q", 2, 5, nc.sync)
            for b in range(B_LOC):
                hs = slice(TQ // 2, TQ)
                nc.sync.dma_start(vs32[:, b, hs], v_res[b][:, hs])
                nc.gpsimd.tensor_copy(v16[:, b, hs, :, 0:D], vs32[:, b, hs])
            load_chunk("q", 5, TQ, nc.sync)

            def prep_transpose(name, ct0, dst, pool=None, n=2):
                """PE-transpose column-tiles [ct0, ct0+n) of q/k into dst.

                2n [128,64]->[64,128] transposes (n column-tiles x 2 batches)
                fill the batch-paired rows of one PSUM tile; a single DVE
                copy drains all n*128 columns to SBUF."""
                pool = pool or pvp
                tp_ps = pool.tile([P, n, P], F16,
                                  tag="st" if pool is stp else "pv",
                                  name=f"tp_{name}{ct0}")
                for i in range(n):
                    t, e = (ct0 + i) // 2, (ct0 + i) % 2
                    # one transpose of the [128, (b,d)=128] slice yields the
                    # batch-paired rows directly (b on the slower free dim)
                    nc.tensor.transpose(tp_ps[:, i, :],
                                        st16s[name][:, t, e], ident[:])
                nc.vector.tensor_copy(dst[:, ct0 * P:(ct0 + n) * P],
                                      tp_ps[:])

            # K^T tiles j0-1 now; the rest are emitted lazily inside the qb0
            # loop (pool allocations are FIFO in emission order, so a transpose
            # emitted before its staging data lands would stall the slot ring).
            # ---- main loop ----
            # stream of S^T tiles: s = qb*32 + 2*j + b
            # groups per qb: [2, 3, 3, ..., 3] (first exp after one j-tile)
            st_tiles = {}      # group -> psum tile
            pt_tiles = {}      # group -> sbuf fp16 tile

            def group_of(s):
                qb_, sl = divmod(s, SPQ)
                if qb_ == NQB - 1:
                    return qb_ * GPQ + min(sl // 3, GPQ - 1), \
                        sl % 3 if sl < 30 else sl - 30
                if sl < 2:
                    return qb_ * GPQ, sl
                return qb_ * GPQ + 1 + (sl - 2) // GROUP, (sl - 2) % GROUP

            def tiles_in_group(g):
                if g // GPQ == NQB - 1:
                    return 2 if g % GPQ == GPQ - 1 else GROUP
                return 2 if g % GPQ == 0 else GROUP

            off_i0 = {}

            def maybe_exp(g):
                """emit exp for group g once all its stream tiles are written.

                ACT path is one instruction.  The offload path emits only the
                PSUM-reading TS here (frees the S^T slot ring); the remaining
                three cheap SBUF ops are emitted ~2 j-steps later via
                finish_off so the in-order DVE queue interleaves other work
                and the chain latency hides under the PV lag."""
                w = tiles_in_group(g) * QB
                if g in OFF_GROUPS:
                    i0 = offp.tile([P, GROUP * QB], I16, tag="i0",
                                   name=f"i0g{g}")
                    nc.vector.tensor_scalar(i0[:, :w], st_tiles[g][:, :w],
                                            EXP_A, EXP_B0,
                                            mybir.AluOpType.mult,
                                            mybir.AluOpType.add)
                    off_i0[g] = i0
                    return
                pt_t = ptp.tile([P, GROUP * QB], F16, tag="pt", name=f"ptg{g}")
                nc.scalar.activation(
                    pt_t[:, :w], st_tiles[g][:, :w],
                    mybir.ActivationFunctionType.Exp, scale=0.125,
                )
                pt_tiles[g] = pt_t

            def finish_off(g):
                w = tiles_in_group(g) * QB
                i0 = off_i0.pop(g)
                pt_t = ptp.tile([P, GROUP * QB], F16, tag="pt", name=f"ptg{g}")
                i1 = offp.tile([P, GROUP * QB], I16, tag="i1", name=f"i1g{g}")
                nc.vector.tensor_scalar(i1[:, :w], i0[:, :w], 512, None,
                                        mybir.AluOpType.subtract)
                t1 = offp.tile([P, GROUP * QB], F16, tag="t1", name=f"t1g{g}")
                nc.vector.tensor_scalar(t1[:, :w], i1[:, :w].bitcast(F16),
                                        SQRT2, None, mybir.AluOpType.mult)
                nc.vector.tensor_tensor(pt_t[:, :w], t1[:, :w],
                                        i0[:, :w].bitcast(F16),
                                        mybir.AluOpType.add)
                pt_tiles[g] = pt_t

            def pt_slice(s, t4):
                """lhsT slice [128 j, 128 q] for stream tile s, q-subtile t4"""
                g, slot = group_of(s)
                c0 = slot * QB + t4 * P
                return pt_tiles[g][:, c0:c0 + P]

            # Q^T tiles for q-block 0
            prep_transpose("q", 0, qt, pool=pvp)
            prep_transpose("q", 2, qt, pool=stp)
            prep_transpose("k", 0, kt, pool=pvp)

            # ---- per-q-block loop ----
            o_acc = {}
            for qb in range(NQB):
                qs = slice(qb * QB, (qb + 1) * QB)
                PV_LAG = 14 if qb == 0 else 7
                pending_off = {}
                for step in range(NT + PV_LAG):
                    if step < NT:
                        j = step
                        if qb == 0 and j % 4 == 0 and j + 2 < NT:
                            prep_transpose("k", j + 2, kt,
                                           n=(4 if j + 6 <= NT else 2))
                        if qb + 1 < NQB and 3 <= j < 5:
                            prep_transpose(
                                "q", (qb + 1) * QTPB + 2 * (j - 3), qt)
                        for b in range(B_LOC):
                            s = qb * SPQ + 2 * j + b
                            g, slot = group_of(s)
                            if slot == 0:
                                st_tiles[g] = stp.tile([P, GROUP * QB], F32,
                                                       tag="st", name=f"stg{g}")
                            rows = slice(b * D, (b + 1) * D)
                            nc.tensor.matmul(
                                st_tiles[g][:, slot * QB:(slot + 1) * QB],
                                kt[rows, j * P:(j + 1) * P],
                                qt[rows, qs],
                                start=True, stop=True,
                            )
                            if slot == tiles_in_group(g) - 1:
                                maybe_exp(g)
                                if g in OFF_GROUPS:
                                    pending_off.setdefault(step + 2,
                                                           []).append(g)
                    for g in pending_off.pop(step, ()):
                        finish_off(g)
                    if step == PV_LAG:
                        for b in range(B_LOC):
                            o_acc[b] = pvp.tile([P, QTPB, D + 1], F32, tag="pv",
                                                name=f"oacc{qb}_{b}")
                    if step >= PV_LAG and step - PV_LAG < NT:
                        j = step - PV_LAG
                        for b in range(B_LOC):
                            s = qb * SPQ + 2 * j + b
                            for t4 in range(QTPB):
                                # start=True zeroes the ENTIRE psum bank, so
                                # only the first matmul into this o_acc bank
                                # may set it; later regions accumulate onto
                                # the already-zeroed bank.
                                nc.tensor.matmul(
                                    o_acc[b][:, t4, :],
                                    pt_slice(s, t4),
                                    v16[:, b, j // 2, j % 2, :],
                                    start=(j == 0 and t4 == 0),
                                    stop=(j == NT - 1),
                                    skip_group_check=True,
                                )
                for b in range(B_LOC):
                    recip4 = ostp.tile([P, QTPB, 1], F32, tag="recip",
                                       name=f"recip{qb}_{b}")
                    nc.vector.reciprocal(recip4[:], o_acc[b][:, :, D:D + 1])
                    o_out = ostp.tile([P, QTPB, D], F32, tag="oo",
                                      name=f"oo{qb}_{b}")
                    nc.vector.tensor_tensor(
                        o_out[:], o_acc[b][:, :, 0:D],
                        recip4[:].to_broadcast((P, QTPB, D)),
                        mybir.AluOpType.mult,
                    )
                    o_dst = o_hbm[b, :, :].rearrange(
                        "(t p e) d -> p t e d", p=P,
                        e=2)[:, 2 * qb:2 * qb + 2]
                    nc.sync.dma_start(o_dst, o_out[:])

    nc.compile()
    return nc


def get_nc():
    global _nc_cache
    if _nc_cache is None:
        _nc_cache = build()
    return _nc_cache


def kernel(queries: np.ndarray, keys: np.ndarray, values: np.ndarray) -> np.ndarray:
    from concourse.bass_utils import run_bass_kernel_spmd

    queries = np.ascontiguousarray(np.asarray(queries, dtype=np.float32))
    keys = np.ascontiguousarray(np.asarray(keys, dtype=np.float32))
    values = np.ascontiguousarray(np.asarray(values, dtype=np.float32))

    nc = get_nc()
    in_maps = []
    for c in range(N_CORES):
        sl = slice(c * B_LOC, (c + 1) * B_LOC)
        in_maps.append({
            "queries": queries[sl],
            "keys": keys[sl],
            "values": values[sl],
        })
    res = run_bass_kernel_spmd(nc, in_maps, core_ids=list(range(N_CORES)))
    return np.concatenate([r["out"] for r in res.results], axis=0)


if __name__ == "__main__":
    rng = np.random.default_rng(0)
    q = rng.standard_normal((B_FULL, N, D), dtype=np.float32)
    k = rng.standard_normal((B_FULL, N, D), dtype=np.float32)
    v = rng.standard_normal((B_FULL, N, D), dtype=np.float32)
    o = kernel(queries=q, keys=k, values=v)
    s = q @ k.transpose(0, 2, 1) / np.sqrt(D)
    w = np.exp(s - s.max(-1, keepdims=True))
    w /= w.sum(-1, keepdims=True)
    ref = w @ v
    err = np.abs(o - ref).max() / np.abs(ref).max()
    print("rel err:", err)


# revision 24
# speedup vs baseline: 1.1497x; 1.0060x over previous
"""Trainium2 Bass kernel: batched dense attention.

Full inputs: queries/keys/values [16, 2048, 64] fp32.
Shards batch dim across 8 NeuronCores (2 batches per core).

Per-core algorithm (batches A, B local):
  S^T[j, q] = K[j, :] . Q[q, :]           (PE, fp16 operands, fp32 PSUM)
  P^T = exp(S^T / 8)                       (ACT exp for most tiles; a tunable
                                            subset is offloaded to DVE+Pool
                                            via a phase-averaged Schraudolph
                                            bit-trick exp, see below)
  O[q, d'] = sum_j P^T[j, q] V'[j, d']     (PE; V' = [V | ones] so col 64 = sums)
  out[q, :] = O[q, 0:64] / O[q, 64]        (DVE reciprocal + broadcast mult)

Engine balance: exp for all 128 stream tiles on ACT alone costs ~64us while
PE needs only ~46us, so 11 of the 44 S^T groups bypass ACT via a DVE chain:
  i0 = round(s*A + B0)  int16   (tensor_scalar, fused convert, exact RNE)
  i1 = i0 - 512         int16   (tensor_scalar, 4x mode)
  t1 = bc16(i1)*sqrt2   fp16    (tensor_scalar, 4x mode)
  p  = t1 + bc16(i0)    fp16    (tensor_tensor, 2x mode)
which computes the average of two phase-shifted Schraudolph exp estimates
(elementwise |rel err| < 0.9%, end-to-end ~2.5e-3 vs the 2e-2 gate).  The
PSUM-reading i0 step is emitted at group completion (frees the S^T ring);
the three cheap SBUF ops are deferred 2 j-steps so the in-order DVE queue
interleaves other work and the chain latency hides under the PV lag.
Engine busy: ACT ~48us, DVE ~47us, PE ~46us, Pool ~12us.

PSUM note: matmul start=True zeroes the ENTIRE psum bank, so the four
interleaved per-t4 accumulation regions of an o_acc bank set start only on
the very first matmul into the bank (see the PV loop).

Layout notes:
  - PV matmuls put q on the OUTPUT PARTITION dim (lhsT = P^T slice, rhs = V'),
    so each matmul's moving free dim is 65 instead of 512: PE time for the PV
    phase halves versus the O^T layout, and the output lands in the natural
    [q, d] layout (no final transposes, short drain tail).
  - Q^T / K^T built by PE transposes of natural tiles, batch-PAIRED so batch A
    lands on SBUF partitions 0-63 and batch B on 64-127.
  - S^T PSUM tiles grouped [2,3,3,...] per q-block so the first exp fires
    after just one j-tile of QK; groups are 3-wide elsewhere to amortize ACT
    per-instruction overhead.
  - Head-critical input casts (first k/q chunks) run on DVE; later casts on
    Pool (GPSIMD) so the in-order DVE queue never holds a cast waiting on
    late DMA data.  k/q/v loads are chunked by first-use and spread across
    the SP and ACT DMA queues to overlap HWDGE issue serialization.
"""

import sys
for _p in ("/opt/trn_rl_repo", "/root/.axon_site/_ro/trn_rl_repo"):
    if _p not in sys.path:
        sys.path.insert(0, _p)

import numpy as np

import concourse.bass as bass
import concourse.mybir as mybir
from concourse import bacc
from concourse.tile import TileContext
from concourse.masks import make_identity

F32 = mybir.dt.float32
F16 = mybir.dt.float16
I16 = mybir.dt.int16
P = 128

N_CORES = 8
B_FULL, N, D = 16, 2048, 64
B_LOC = B_FULL // N_CORES          # 2 batches per core
NT = N // P                        # 16 tiles of 128 along q and j
TQ = NT // 2                       # 8 pair-interleaved staging tiles of 256
QB = 512                           # q-block (PSUM bank width in fp32)
NQB = N // QB                      # 4 q-blocks
QTPB = QB // P                     # 4 q-tiles per q-block
GROUP = 3                          # S^T stream tiles per exp instruction
SPQ = 2 * NT                       # 32 stream tiles per q-block
GPQ = 11                           # groups per q-block: [2,3,3,...,3]

# Schraudolph constants: exp(x*0.125) ~ avg of 2 phase-shifted estimates
EXP_A = 0.125 * 1024 * 1.4426950408889634          # 184.6649...
EXP_C = 56
EXP_B0 = float(15 * 1024 - EXP_C - 1024)           # nphase=2: fold the /2
SQRT2 = 1.4142135623730951

# which groups (local index within q-block) use the DVE/Pool offload path
OFF_LOCAL = {0: (5, 8), 1: (2, 5, 8), 2: (2, 5, 8), 3: (2, 5, 7)}
OFF_GROUPS = frozenset(qb * GPQ + g for qb, gs in OFF_LOCAL.items() for g in gs)

_nc_cache = None


def build():
    nc = bacc.Bacc(None, target_bir_lowering=False)
    q_hbm = nc.dram_tensor("queries", [B_LOC, N, D], F32, kind="ExternalInput")
    k_hbm = nc.dram_tensor("keys", [B_LOC, N, D], F32, kind="ExternalInput")
    v_hbm = nc.dram_tensor("values", [B_LOC, N, D], F32, kind="ExternalInput")
    o_hbm = nc.dram_tensor("out", [B_LOC, N, D], F32, kind="ExternalOutput")

    with TileContext(nc) as tc:
        with (
            tc.tile_pool(name="cst", bufs=1) as cst,
            tc.tile_pool(name="stage", bufs=2) as stage,
            tc.tile_pool(name="persist", bufs=1) as persist,
            tc.tile_pool(name="pt", bufs=14) as ptp,
            tc.tile_pool(name="off", bufs=2) as offp,
            tc.tile_pool(name="ost", bufs=4) as ostp,
            tc.tile_pool(name="st", bufs=2, space="PSUM") as stp,
            tc.tile_pool(name="pv", bufs=2, space="PSUM") as pvp,
        ):
            ident = cst.tile([P, P], F16)
            make_identity(nc, ident)

            # ---- persistent SBUF buffers ----
            # Q^T / K^T, batch-paired: rows 0-63 batch A (d), 64-127 batch B.
            qt = persist.tile([P, N], F16, tag="qt")
            kt = persist.tile([P, N], F16, tag="kt")
            # V' = [V | ones]: [128 j, b, t, e, 65] fp16 (pair-interleaved)
            v16 = persist.tile([P, B_LOC, TQ, 2, D + 1], F16, tag="v16")

            # Pair-interleaved staging: partition p of staged tile t holds the
            # TWO consecutive rows 256t+2p / 256t+2p+1 (e dim), so every DMA
            # descriptor is a 512B contiguous run (full bus efficiency; a 256B
            # run is charged 2x).  Row index within column-tile ct = 2t+e is a
            # fixed permutation shared by K and V (and by Q and the output
            # store), so attention math is unaffected.
            q_res = [q_hbm[b, :, :].rearrange("(t p e) d -> p t e d", p=P, e=2)
                     for b in range(B_LOC)]
            k_res = [k_hbm[b, :, :].rearrange("(t p e) d -> p t e d", p=P, e=2)
                     for b in range(B_LOC)]

            st32s, st16s = {}, {}
            for name in ("k", "q"):
                st32s[name] = stage.tile([P, TQ, B_LOC, 2, D], F32,
                                         tag=f"{name}s32", name=f"{name}s32")
                # transpose-friendly layout: for fixed (t, e) the (b, d)
                # block is one contiguous 128-wide free dim
                st16s[name] = stage.tile([P, TQ, 2, B_LOC, D], F16,
                                         tag=f"{name}s16", name=f"{name}s16")
            vs32 = stage.tile([P, B_LOC, TQ, 2, D], F32, tag="vs32")

            def load_chunk(name, t0, t1, eng, cast_eng=None):
                re_aps = k_res if name == "k" else q_res
                for b in range(B_LOC):
                    eng.dma_start(st32s[name][:, t0:t1, b],
                                  re_aps[b][:, t0:t1])
                    (cast_eng or nc.gpsimd).tensor_copy(
                        st16s[name][:, t0:t1, :, b, :],
                        st32s[name][:, t0:t1, b])

            # Loads ordered by need; first k/q chunks on the ACT queue, the
            # rest on SP so the issue paths overlap (HWDGE itself is serial).
            # Head-critical chunks cast on DVE; everything later casts on
            # the otherwise-idle Pool engine so the in-order DVE queue never
            # holds a cast that waits on late DMA data (head-of-line).
            load_chunk("q", 0, 2, nc.sync, cast_eng=nc.vector)
            load_chunk("k", 0, 2, nc.scalar, cast_eng=nc.vector)
            load_chunk("k", 2, 5, nc.scalar, cast_eng=nc.vector)
            load_chunk("k", 5, TQ, nc.scalar, cast_eng=nc.vector)
            # V: cast into v16[..., :64] (per batch-half), ones col
            nc.gpsimd.memset(v16[:, :, :, :, D:D + 1], 1.0)
            v_res = [v_hbm[b, :, :].rearrange("(t p e) d -> p t e d", p=P, e=2)
                     for b in range(B_LOC)]
            for b in range(B_LOC):
                hs = slice(0, TQ // 2)
                nc.sync.dma_start(vs32[:, b, hs], v_res[b][:, hs])
                nc.gpsimd.tensor_copy(v16[:, b, hs, :, 0:D], vs32[:, b, hs])
            load_chunk("q", 2, 5, nc.sync)
            for b in range(B_LOC):
                hs = slice(TQ // 2, TQ)
                nc.sync.dma_start(vs32[:, b, hs], v_res[b][:, hs])
                nc.gpsimd.tensor_copy(v16[:, b, hs, :, 0:D], vs32[:, b, hs])
            load_chunk("q", 5, TQ, nc.sync)

            def prep_transpose(name, ct0, dst, pool=None, n=2):
                """PE-transpose column-tiles [ct0, ct0+n) of q/k into dst.

                2n [128,64]->[64,128] transposes (n column-tiles x 2 batches)
                fill the batch-paired rows of one PSUM tile; a single DVE
                copy drains all n*128 columns to SBUF."""
                pool = pool or pvp
                tp_ps = pool.tile([P, n, P], F16,
                                  tag="st" if pool is stp else "pv",
                                  name=f"tp_{name}{ct0}")
                for i in range(n):
                    t, e = (ct0 + i) // 2, (ct0 + i) % 2
                    # one transpose of the [128, (b,d)=128] slice yields the
                    # batch-paired rows directly (b on the slower free dim)
                    nc.tensor.transpose(tp_ps[:, i, :],
                                        st16s[name][:, t, e], ident[:])
                nc.vector.tensor_copy(dst[:, ct0 * P:(ct0 + n) * P],
                                      tp_ps[:])

            # K^T tiles j0-1 now; the rest are emitted lazily inside the qb0
            # loop (pool allocations are FIFO in emission order, so a transpose
            # emitted before its staging data lands would stall the slot ring).
            # ---- main loop ----
            # stream of S^T tiles: s = qb*32 + 2*j + b
            # groups per qb: [2, 3, 3, ..., 3] (first exp after one j-tile)
            st_tiles = {}      # group -> psum tile
            pt_tiles = {}      # group -> sbuf fp16 tile

            def group_of(s):
                qb_, sl = divmod(s, SPQ)
                if qb_ == NQB - 1:
                    return qb_ * GPQ + min(sl // 3, GPQ - 1), \
                        sl % 3 if sl < 30 else sl - 30
                if sl < 2:
                    return qb_ * GPQ, sl
                return qb_ * GPQ + 1 + (sl - 2) // GROUP, (sl - 2) % GROUP

            def tiles_in_group(g):
                if g // GPQ == NQB - 1:
                    return 2 if g % GPQ == GPQ - 1 else GROUP
                return 2 if g % GPQ == 0 else GROUP

            off_i0 = {}

            def maybe_exp(g):
                """emit exp for group g once all its stream tiles are written.

                ACT path is one instruction.  The offload path emits only the
                PSUM-reading TS here (frees the S^T slot ring); the remaining
                three cheap SBUF ops are emitted ~2 j-steps later via
                finish_off so the in-order DVE queue interleaves other work
                and the chain latency hides under the PV lag."""
                w = tiles_in_group(g) * QB
                if g in OFF_GROUPS:
                    i0 = offp.tile([P, GROUP * QB], I16, tag="i0",
                                   name=f"i0g{g}")
                    nc.vector.tensor_scalar(i0[:, :w], st_tiles[g][:, :w],
                                            EXP_A, EXP_B0,
                                            mybir.AluOpType.mult,
                                            mybir.AluOpType.add)
                    off_i0[g] = i0
                    return
                pt_t = ptp.tile([P, GROUP * QB], F16, tag="pt", name=f"ptg{g}")
                nc.scalar.activation(
                    pt_t[:, :w], st_tiles[g][:, :w],
                    mybir.ActivationFunctionType.Exp, scale=0.125,
                )
                pt_tiles[g] = pt_t

            def finish_off(g):
                w = tiles_in_group(g) * QB
                i0 = off_i0.pop(g)
                pt_t = ptp.tile([P, GROUP * QB], F16, tag="pt", name=f"ptg{g}")
                i1 = offp.tile([P, GROUP * QB], I16, tag="i1", name=f"i1g{g}")
                nc.vector.tensor_scalar(i1[:, :w], i0[:, :w], 512, None,
                                        mybir.AluOpType.subtract)
                t1 = offp.tile([P, GROUP * QB], F16, tag="t1", name=f"t1g{g}")
                nc.vector.tensor_scalar(t1[:, :w], i1[:, :w].bitcast(F16),
                                        SQRT2, None, mybir.AluOpType.mult)
                nc.vector.tensor_tensor(pt_t[:, :w], t1[:, :w],
                                        i0[:, :w].bitcast(F16),
                                        mybir.AluOpType.add)
                pt_tiles[g] = pt_t

            def pt_slice(s, t4):
                """lhsT slice [128 j, 128 q] for stream tile s, q-subtile t4"""
                g, slot = group_of(s)
                c0 = slot * QB + t4 * P
                return pt_tiles[g][:, c0:c0 + P]

            # Q^T tiles for q-block 0
            prep_transpose("q", 0, qt, pool=pvp)
            prep_transpose("q", 2, qt, pool=stp)
            prep_transpose("k", 0, kt, pool=pvp)

            # ---- per-q-block loop ----
            o_acc = {}
            for qb in range(NQB):
                qs = slice(qb * QB, (qb + 1) * QB)
                PV_LAG = 14 if qb == 0 else 7
                pending_off = {}
                for step in range(NT + PV_LAG):
                    if step < NT:
                        j = step
                        if qb == 0 and j in (0, 8):
                            prep_transpose("k", j + 2, kt,
                                           n=(8 if j == 0 else 6))
                        if qb + 1 < NQB and 3 <= j < 5:
                            prep_transpose(
                                "q", (qb + 1) * QTPB + 2 * (j - 3), qt)
                        for b in range(B_LOC):
                            s = qb * SPQ + 2 * j + b
                            g, slot = group_of(s)
                            if slot == 0:
                                st_tiles[g] = stp.tile([P, GROUP * QB], F32,
                                                       tag="st", name=f"stg{g}")
                            rows = slice(b * D, (b + 1) * D)
                            nc.tensor.matmul(
                                st_tiles[g][:, slot * QB:(slot + 1) * QB],
                                kt[rows, j * P:(j + 1) * P],
                                qt[rows, qs],
                                start=True, stop=True,
                            )
                            if slot == tiles_in_group(g) - 1:
                                maybe_exp(g)
                                if g in OFF_GROUPS:
                                    pending_off.setdefault(step + 2,
                                                           []).append(g)
                    for g in pending_off.pop(step, ()):
                        finish_off(g)
                    if step == PV_LAG:
                        for b in range(B_LOC):
                            o_acc[b] = pvp.tile([P, QTPB, D + 1], F32, tag="pv",
                                                name=f"oacc{qb}_{b}")
                    if step >= PV_LAG and step - PV_LAG < NT:
                        j = step - PV_LAG
                        for b in range(B_LOC):
                            s = qb * SPQ + 2 * j + b
                            for t4 in range(QTPB):
                                # start=True zeroes the ENTIRE psum bank, so
                                # only the first matmul into this o_acc bank
                                # may set it; later regions accumulate onto
                                # the already-zeroed bank.
                                nc.tensor.matmul(
                                    o_acc[b][:, t4, :],
                                    pt_slice(s, t4),
                                    v16[:, b, j // 2, j % 2, :],
                                    start=(j == 0 and t4 == 0),
                                    stop=(j == NT - 1),
                                    skip_group_check=True,
                                )
                for b in range(B_LOC):
                    recip4 = ostp.tile([P, QTPB, 1], F32, tag="recip",
                                       name=f"recip{qb}_{b}")
                    nc.vector.reciprocal(recip4[:], o_acc[b][:, :, D:D + 1])
                    o_out = ostp.tile([P, QTPB, D], F32, tag="oo",
                                      name=f"oo{qb}_{b}")
                    nc.vector.tensor_tensor(
                        o_out[:], o_acc[b][:, :, 0:D],
                        recip4[:].to_broadcast((P, QTPB, D)),
                        mybir.AluOpType.mult,
                    )
                    o_dst = o_hbm[b, :, :].rearrange(
                        "(t p e) d -> p t e d", p=P,
                        e=2)[:, 2 * qb:2 * qb + 2]
                    nc.sync.dma_start(o_dst, o_out[:])

    nc.compile()
    return nc


def get_nc():
    global _nc_cache
    if _nc_cache is None:
        _nc_cache = build()
    return _nc_cache


def kernel(queries: np.ndarray, keys: np.ndarray, values: np.ndarray) -> np.ndarray:
    from concourse.bass_utils import run_bass_kernel_spmd

    queries = np.ascontiguousarray(np.asarray(queries, dtype=np.float32))
    keys = np.ascontiguousarray(np.asarray(keys, dtype=np.float32))
    values = np.ascontiguousarray(np.asarray(values, dtype=np.float32))

    nc = get_nc()
    in_maps = []
    for c in range(N_CORES):
        sl = slice(c * B_LOC, (c + 1) * B_LOC)
        in_maps.append({
            "queries": queries[sl],
            "keys": keys[sl],
            "values": values[sl],
        })
    res = run_bass_kernel_spmd(nc, in_maps, core_ids=list(range(N_CORES)))
    return np.concatenate([r["out"] for r in res.results], axis=0)


if __name__ == "__main__":
    rng = np.random.default_rng(0)
    q = rng.standard_normal((B_FULL, N, D), dtype=np.float32)
    k = rng.standard_normal((B_FULL, N, D), dtype=np.float32)
    v = rng.standard_normal((B_FULL, N, D), dtype=np.float32)
    o = kernel(queries=q, keys=k, values=v)
    s = q @ k.transpose(0, 2, 1) / np.sqrt(D)
    w = np.exp(s - s.max(-1, keepdims=True))
    w /= w.sum(-1, keepdims=True)
    ref = w @ v
    err = np.abs(o - ref).max() / np.abs(ref).max()
    print("rel err:", err)


# revision 25
# speedup vs baseline: 1.1556x; 1.0051x over previous
"""Trainium2 Bass kernel: batched dense attention.

Full inputs: queries/keys/values [16, 2048, 64] fp32.
Shards batch dim across 8 NeuronCores (2 batches per core).

Per-core algorithm (batches A, B local):
  S^T[j, q] = K[j, :] . Q[q, :]           (PE, fp16 operands, fp32 PSUM)
  P^T = exp(S^T / 8)                       (ACT exp for most tiles; a tunable
                                            subset is offloaded to DVE+Pool
                                            via a phase-averaged Schraudolph
                                            bit-trick exp, see below)
  O[q, d'] = sum_j P^T[j, q] V'[j, d']     (PE; V' = [V | ones] so col 64 = sums)
  out[q, :] = O[q, 0:64] / O[q, 64]        (DVE reciprocal + broadcast mult)

Engine balance: exp for all 128 stream tiles on ACT alone costs ~64us while
PE needs only ~46us, so 11 of the 44 S^T groups bypass ACT via a DVE chain:
  i0 = round(s*A + B0)  int16   (tensor_scalar, fused convert, exact RNE)
  i1 = i0 - 512         int16   (tensor_scalar, 4x mode)
  t1 = bc16(i1)*sqrt2   fp16    (tensor_scalar, 4x mode)
  p  = t1 + bc16(i0)    fp16    (tensor_tensor, 2x mode)
which computes the average of two phase-shifted Schraudolph exp estimates
(elementwise |rel err| < 0.9%, end-to-end ~2.5e-3 vs the 2e-2 gate).  The
PSUM-reading i0 step is emitted at group completion (frees the S^T ring);
the three cheap SBUF ops are deferred 2 j-steps so the in-order DVE queue
interleaves other work and the chain latency hides under the PV lag.
Engine busy: ACT ~48us, DVE ~47us, PE ~46us, Pool ~12us.

PSUM note: matmul start=True zeroes the ENTIRE psum bank, so the four
interleaved per-t4 accumulation regions of an o_acc bank set start only on
the very first matmul into the bank (see the PV loop).

Layout notes:
  - PV matmuls put q on the OUTPUT PARTITION dim (lhsT = P^T slice, rhs = V'),
    so each matmul's moving free dim is 65 instead of 512: PE time for the PV
    phase halves versus the O^T layout, and the output lands in the natural
    [q, d] layout (no final transposes, short drain tail).
  - Q^T / K^T built by PE transposes of natural tiles, batch-PAIRED so batch A
    lands on SBUF partitions 0-63 and batch B on 64-127.
  - S^T PSUM tiles grouped [2,3,3,...] per q-block so the first exp fires
    after just one j-tile of QK; groups are 3-wide elsewhere to amortize ACT
    per-instruction overhead.
  - Head-critical input casts (first k/q chunks) run on DVE; later casts on
    Pool (GPSIMD) so the in-order DVE queue never holds a cast waiting on
    late DMA data.  k/q/v loads are chunked by first-use and spread across
    the SP and ACT DMA queues to overlap HWDGE issue serialization.
"""

import sys
for _p in ("/opt/trn_rl_repo", "/root/.axon_site/_ro/trn_rl_repo"):
    if _p not in sys.path:
        sys.path.insert(0, _p)

import numpy as np

import concourse.bass as bass
import concourse.mybir as mybir
from concourse import bacc
from concourse.tile import TileContext
from concourse.masks import make_identity

F32 = mybir.dt.float32
F16 = mybir.dt.float16
I16 = mybir.dt.int16
P = 128

N_CORES = 8
B_FULL, N, D = 16, 2048, 64
B_LOC = B_FULL // N_CORES          # 2 batches per core
NT = N // P                        # 16 tiles of 128 along q and j
TQ = NT // 2                       # 8 pair-interleaved staging tiles of 256
QB = 512                           # q-block (PSUM bank width in fp32)
NQB = N // QB                      # 4 q-blocks
QTPB = QB // P                     # 4 q-tiles per q-block
GROUP = 3                          # S^T stream tiles per exp instruction
SPQ = 2 * NT                       # 32 stream tiles per q-block
GPQ = 11                           # groups per q-block: [2,3,3,...,3]

# Schraudolph constants: exp(x*0.125) ~ UNWEIGHTED sum of 2 phase-shifted
# estimates p = bc16(i0) + bc16(i0-512); the (1 + 1/sqrt2) scale and the
# optimal sawtooth offset fold into the bias (C=842), so no fp multiply is
# needed (elementwise |rel err| 1.11% vs 0.87% for the weighted average).
EXP_A = 0.125 * 1024 * 1.4426950408889634          # 184.6649...
EXP_B0 = float(15 * 1024 - 842)

# which groups (local index within q-block) use the DVE/Pool offload path
OFF_LOCAL = {0: (2, 5, 8), 1: (2, 5, 8), 2: (2, 5, 8), 3: (2, 5, 7)}
OFF_GROUPS = frozenset(qb * GPQ + g for qb, gs in OFF_LOCAL.items() for g in gs)

_nc_cache = None


def build():
    nc = bacc.Bacc(None, target_bir_lowering=False)
    q_hbm = nc.dram_tensor("queries", [B_LOC, N, D], F32, kind="ExternalInput")
    k_hbm = nc.dram_tensor("keys", [B_LOC, N, D], F32, kind="ExternalInput")
    v_hbm = nc.dram_tensor("values", [B_LOC, N, D], F32, kind="ExternalInput")
    o_hbm = nc.dram_tensor("out", [B_LOC, N, D], F32, kind="ExternalOutput")

    with TileContext(nc) as tc:
        with (
            tc.tile_pool(name="cst", bufs=1) as cst,
            tc.tile_pool(name="stage", bufs=2) as stage,
            tc.tile_pool(name="persist", bufs=1) as persist,
            tc.tile_pool(name="pt", bufs=14) as ptp,
            tc.tile_pool(name="off", bufs=2) as offp,
            tc.tile_pool(name="ost", bufs=4) as ostp,
            tc.tile_pool(name="st", bufs=2, space="PSUM") as stp,
            tc.tile_pool(name="pv", bufs=2, space="PSUM") as pvp,
        ):
            ident = cst.tile([P, P], F16)
            make_identity(nc, ident)

            # ---- persistent SBUF buffers ----
            # Q^T / K^T, batch-paired: rows 0-63 batch A (d), 64-127 batch B.
            qt = persist.tile([P, N], F16, tag="qt")
            kt = persist.tile([P, N], F16, tag="kt")
            # V' = [V | ones]: [128 j, b, t, e, 65] fp16 (pair-interleaved)
            v16 = persist.tile([P, B_LOC, TQ, 2, D + 1], F16, tag="v16")

            # Pair-interleaved staging: partition p of staged tile t holds the
            # TWO consecutive rows 256t+2p / 256t+2p+1 (e dim), so every DMA
            # descriptor is a 512B contiguous run (full bus efficiency; a 256B
            # run is charged 2x).  Row index within column-tile ct = 2t+e is a
            # fixed permutation shared by K and V (and by Q and the output
            # store), so attention math is unaffected.
            q_res = [q_hbm[b, :, :].rearrange("(t p e) d -> p t e d", p=P, e=2)
                     for b in range(B_LOC)]
            k_res = [k_hbm[b, :, :].rearrange("(t p e) d -> p t e d", p=P, e=2)
                     for b in range(B_LOC)]

            st32s, st16s = {}, {}
            for name in ("k", "q"):
                st32s[name] = stage.tile([P, TQ, B_LOC, 2, D], F32,
                                         tag=f"{name}s32", name=f"{name}s32")
                # transpose-friendly layout: for fixed (t, e) the (b, d)
                # block is one contiguous 128-wide free dim
                st16s[name] = stage.tile([P, TQ, 2, B_LOC, D], F16,
                                         tag=f"{name}s16", name=f"{name}s16")
            vs32 = stage.tile([P, B_LOC, TQ, 2, D], F32, tag="vs32")

            def load_chunk(name, t0, t1, eng, cast_eng=None):
                re_aps = k_res if name == "k" else q_res
                for b in range(B_LOC):
                    eng.dma_start(st32s[name][:, t0:t1, b],
                                  re_aps[b][:, t0:t1])
                    (cast_eng or nc.gpsimd).tensor_copy(
                        st16s[name][:, t0:t1, :, b, :],
                        st32s[name][:, t0:t1, b])

            # Loads ordered by need; first k/q chunks on the ACT queue, the
            # rest on SP so the issue paths overlap (HWDGE itself is serial).
            # Head-critical chunks cast on DVE; everything later casts on
            # the otherwise-idle Pool engine so the in-order DVE queue never
            # holds a cast that waits on late DMA data (head-of-line).
            load_chunk("q", 0, 2, nc.sync, cast_eng=nc.vector)
            load_chunk("k", 0, 2, nc.scalar, cast_eng=nc.vector)
            load_chunk("k", 2, 5, nc.scalar, cast_eng=nc.vector)
            load_chunk("k", 5, TQ, nc.scalar, cast_eng=nc.vector)
            # V: cast into v16[..., :64] (per batch-half), ones col
            nc.gpsimd.memset(v16[:, :, :, :, D:D + 1], 1.0)
            v_res = [v_hbm[b, :, :].rearrange("(t p e) d -> p t e d", p=P, e=2)
                     for b in range(B_LOC)]
            for b in range(B_LOC):
                hs = slice(0, TQ // 2)
                nc.sync.dma_start(vs32[:, b, hs], v_res[b][:, hs])
                nc.gpsimd.tensor_copy(v16[:, b, hs, :, 0:D], vs32[:, b, hs])
            load_chunk("q", 2, 5, nc.sync)
            for b in range(B_LOC):
                hs = slice(TQ // 2, TQ)
                nc.sync.dma_start(vs32[:, b, hs], v_res[b][:, hs])
                nc.gpsimd.tensor_copy(v16[:, b, hs, :, 0:D], vs32[:, b, hs])
            load_chunk("q", 5, TQ, nc.sync)

            def prep_transpose(name, ct0, dst, pool=None, n=2):
                """PE-transpose column-tiles [ct0, ct0+n) of q/k into dst.

                2n [128,64]->[64,128] transposes (n column-tiles x 2 batches)
                fill the batch-paired rows of one PSUM tile; a single DVE
                copy drains all n*128 columns to SBUF."""
                pool = pool or pvp
                tp_ps = pool.tile([P, n, P], F16,
                                  tag="st" if pool is stp else "pv",
                                  name=f"tp_{name}{ct0}")
                for i in range(n):
                    t, e = (ct0 + i) // 2, (ct0 + i) % 2
                    # one transpose of the [128, (b,d)=128] slice yields the
                    # batch-paired rows directly (b on the slower free dim)
                    nc.tensor.transpose(tp_ps[:, i, :],
                                        st16s[name][:, t, e], ident[:])
                nc.vector.tensor_copy(dst[:, ct0 * P:(ct0 + n) * P],
                                      tp_ps[:])

            # K^T tiles j0-1 now; the rest are emitted lazily inside the qb0
            # loop (pool allocations are FIFO in emission order, so a transpose
            # emitted before its staging data lands would stall the slot ring).
            # ---- main loop ----
            # stream of S^T tiles: s = qb*32 + 2*j + b
            # groups per qb: [2, 3, 3, ..., 3] (first exp after one j-tile)
            st_tiles = {}      # group -> psum tile
            pt_tiles = {}      # group -> sbuf fp16 tile

            def group_of(s):
                qb_, sl = divmod(s, SPQ)
                if qb_ == NQB - 1:
                    return qb_ * GPQ + min(sl // 3, GPQ - 1), \
                        sl % 3 if sl < 30 else sl - 30
                if sl < 2:
                    return qb_ * GPQ, sl
                return qb_ * GPQ + 1 + (sl - 2) // GROUP, (sl - 2) % GROUP

            def tiles_in_group(g):
                if g // GPQ == NQB - 1:
                    return 2 if g % GPQ == GPQ - 1 else GROUP
                return 2 if g % GPQ == 0 else GROUP

            off_i0 = {}

            def maybe_exp(g):
                """emit exp for group g once all its stream tiles are written.

                ACT path is one instruction.  The offload path emits only the
                PSUM-reading TS here (frees the S^T slot ring); the remaining
                three cheap SBUF ops are emitted ~2 j-steps later via
                finish_off so the in-order DVE queue interleaves other work
                and the chain latency hides under the PV lag."""
                w = tiles_in_group(g) * QB
                if g in OFF_GROUPS:
                    i0 = offp.tile([P, GROUP * QB], I16, tag="i0",
                                   name=f"i0g{g}")
                    nc.vector.tensor_scalar(i0[:, :w], st_tiles[g][:, :w],
                                            EXP_A, EXP_B0,
                                            mybir.AluOpType.mult,
                                            mybir.AluOpType.add)
                    off_i0[g] = i0
                    return
                pt_t = ptp.tile([P, GROUP * QB], F16, tag="pt", name=f"ptg{g}")
                nc.scalar.activation(
                    pt_t[:, :w], st_tiles[g][:, :w],
                    mybir.ActivationFunctionType.Exp, scale=0.125,
                )
                pt_tiles[g] = pt_t

            def finish_off(g):
                w = tiles_in_group(g) * QB
                i0 = off_i0.pop(g)
                pt_t = ptp.tile([P, GROUP * QB], F16, tag="pt", name=f"ptg{g}")
                i1 = offp.tile([P, GROUP * QB], I16, tag="i1", name=f"i1g{g}")
                nc.vector.tensor_scalar(i1[:, :w], i0[:, :w], 512, None,
                                        mybir.AluOpType.subtract)
                nc.vector.tensor_tensor(pt_t[:, :w], i1[:, :w].bitcast(F16),
                                        i0[:, :w].bitcast(F16),
                                        mybir.AluOpType.add)
                pt_tiles[g] = pt_t

            def pt_slice(s, t4):
                """lhsT slice [128 j, 128 q] for stream tile s, q-subtile t4"""
                g, slot = group_of(s)
                c0 = slot * QB + t4 * P
                return pt_tiles[g][:, c0:c0 + P]

            # Q^T tiles for q-block 0
            prep_transpose("q", 0, qt, pool=pvp)
            prep_transpose("q", 2, qt, pool=stp)
            prep_transpose("k", 0, kt, pool=pvp)

            # ---- per-q-block loop ----
            o_acc = {}
            for qb in range(NQB):
                qs = slice(qb * QB, (qb + 1) * QB)
                PV_LAG = 14 if qb == 0 else 7
                pending_off = {}
                for step in range(NT + PV_LAG):
                    if step < NT:
                        j = step
                        if qb == 0 and j in (0, 8):
                            prep_transpose("k", j + 2, kt,
                                           n=(8 if j == 0 else 6))
                        if qb + 1 < NQB and 3 <= j < 5:
                            prep_transpose(
                                "q", (qb + 1) * QTPB + 2 * (j - 3), qt)
                        for b in range(B_LOC):
                            s = qb * SPQ + 2 * j + b
                            g, slot = group_of(s)
                            if slot == 0:
                                st_tiles[g] = stp.tile([P, GROUP * QB], F32,
                                                       tag="st", name=f"stg{g}")
                            rows = slice(b * D, (b + 1) * D)
                            nc.tensor.matmul(
                                st_tiles[g][:, slot * QB:(slot + 1) * QB],
                                kt[rows, j * P:(j + 1) * P],
                                qt[rows, qs],
                                start=True, stop=True,
                            )
                            if slot == tiles_in_group(g) - 1:
                                maybe_exp(g)
                                if g in OFF_GROUPS:
                                    pending_off.setdefault(step + 2,
                                                           []).append(g)
                    for g in pending_off.pop(step, ()):
                        finish_off(g)
                    if step == PV_LAG:
                        for b in range(B_LOC):
                            o_acc[b] = pvp.tile([P, QTPB, D + 1], F32, tag="pv",
                                                name=f"oacc{qb}_{b}")
                    if step >= PV_LAG and step - PV_LAG < NT:
                        j = step - PV_LAG
                        for b in range(B_LOC):
                            s = qb * SPQ + 2 * j + b
                            for t4 in range(QTPB):
                                # start=True zeroes the ENTIRE psum bank, so
                                # only the first matmul into this o_acc bank
                                # may set it; later regions accumulate onto
                                # the already-zeroed bank.
                                nc.tensor.matmul(
                                    o_acc[b][:, t4, :],
                                    pt_slice(s, t4),
                                    v16[:, b, j // 2, j % 2, :],
                                    start=(j == 0 and t4 == 0),
                                    stop=(j == NT - 1),
                                    skip_group_check=True,
                                )
                for b in range(B_LOC):
                    recip4 = ostp.tile([P, QTPB, 1], F32, tag="recip",
                                       name=f"recip{qb}_{b}")
                    nc.vector.reciprocal(recip4[:], o_acc[b][:, :, D:D + 1])
                    o_out = ostp.tile([P, QTPB, D], F32, tag="oo",
                                      name=f"oo{qb}_{b}")
                    nc.vector.tensor_tensor(
                        o_out[:], o_acc[b][:, :, 0:D],
                        recip4[:].to_broadcast((P, QTPB, D)),
                        mybir.AluOpType.mult,
                    )
                    o_dst = o_hbm[b, :, :].rearrange(
                        "(t p e) d -> p t e d", p=P,
                        e=2)[:, 2 * qb:2 * qb + 2]
                    nc.sync.dma_start(o_dst, o_out[:])

    nc.compile()
    return nc


def get_nc():
    global _nc_cache
    if _nc_cache is None:
        _nc_cache = build()
    return _nc_cache


def kernel(queries: np.ndarray, keys: np.ndarray, values: np.ndarray) -> np.ndarray:
    from concourse.bass_utils import run_bass_kernel_spmd

    queries = np.ascontiguousarray(np.asarray(queries, dtype=np.float32))
    keys = np.ascontiguousarray(np.asarray(keys, dtype=np.float32))
    values = np.ascontiguousarray(np.asarray(values, dtype=np.float32))

    nc = get_nc()
    in_maps = []
    for c in range(N_CORES):
        sl = slice(c * B_LOC, (c + 1) * B_LOC)
        in_maps.append({
            "queries": queries[sl],
            "keys": keys[sl],
            "values": values[sl],
        })
    res = run_bass_kernel_spmd(nc, in_maps, core_ids=list(range(N_CORES)))
    return np.concatenate([r["out"] for r in res.results], axis=0)


if __name__ == "__main__":
    rng = np.random.default_rng(0)
    q = rng.standard_normal((B_FULL, N, D), dtype=np.float32)
    k = rng.standard_normal((B_FULL, N, D), dtype=np.float32)
    v = rng.standard_normal((B_FULL, N, D), dtype=np.float32)
    o = kernel(queries=q, keys=k, values=v)
    s = q @ k.transpose(0, 2, 1) / np.sqrt(D)
    w = np.exp(s - s.max(-1, keepdims=True))
    w /= w.sum(-1, keepdims=True)
    ref = w @ v
    err = np.abs(o - ref).max() / np.abs(ref).max()
    print("rel err:", err)


# revision 26
# speedup vs baseline: 1.1738x; 1.0158x over previous
"""Trainium2 Bass kernel: batched dense attention.

Full inputs: queries/keys/values [16, 2048, 64] fp32.
Shards batch dim across 8 NeuronCores (2 batches per core).

Per-core algorithm (batches A, B local):
  S^T[j, q] = K[j, :] . Q[q, :]           (PE, fp16 operands, fp32 PSUM)
  P^T = exp(S^T / 8)                       (ACT exp for most tiles; a tunable
                                            subset is offloaded to DVE+Pool
                                            via a phase-averaged Schraudolph
                                            bit-trick exp, see below)
  O[q, d'] = sum_j P^T[j, q] V'[j, d']     (PE; V' = [V | ones] so col 64 = sums)
  out[q, :] = O[q, 0:64] / O[q, 64]        (DVE reciprocal + broadcast mult)

Engine balance: exp for all 128 stream tiles on ACT alone costs ~64us while
PE needs only ~46us, so 11 of the 44 S^T groups bypass ACT via a DVE chain:
  i0 = round(s*A + B0)  int16   (tensor_scalar, fused convert, exact RNE)
  i1 = i0 - 512         int16   (tensor_scalar, 4x mode)
  t1 = bc16(i1)*sqrt2   fp16    (tensor_scalar, 4x mode)
  p  = t1 + bc16(i0)    fp16    (tensor_tensor, 2x mode)
which computes the average of two phase-shifted Schraudolph exp estimates
(elementwise |rel err| < 0.9%, end-to-end ~2.5e-3 vs the 2e-2 gate).  The
PSUM-reading i0 step is emitted at group completion (frees the S^T ring);
the three cheap SBUF ops are deferred 2 j-steps so the in-order DVE queue
interleaves other work and the chain latency hides under the PV lag.
Engine busy: ACT ~48us, DVE ~47us, PE ~46us, Pool ~12us.

PSUM note: matmul start=True zeroes the ENTIRE psum bank, so the four
interleaved per-t4 accumulation regions of an o_acc bank set start only on
the very first matmul into the bank (see the PV loop).

Layout notes:
  - PV matmuls put q on the OUTPUT PARTITION dim (lhsT = P^T slice, rhs = V'),
    so each matmul's moving free dim is 65 instead of 512: PE time for the PV
    phase halves versus the O^T layout, and the output lands in the natural
    [q, d] layout (no final transposes, short drain tail).
  - Q^T / K^T built by PE transposes of natural tiles, batch-PAIRED so batch A
    lands on SBUF partitions 0-63 and batch B on 64-127.
  - S^T PSUM tiles grouped [2,3,3,...] per q-block so the first exp fires
    after just one j-tile of QK; groups are 3-wide elsewhere to amortize ACT
    per-instruction overhead.
  - Head-critical input casts (first k/q chunks) run on DVE; later casts on
    Pool (GPSIMD) so the in-order DVE queue never holds a cast waiting on
    late DMA data.  k/q/v loads are chunked by first-use and spread across
    the SP and ACT DMA queues to overlap HWDGE issue serialization.
"""

import sys
for _p in ("/opt/trn_rl_repo", "/root/.axon_site/_ro/trn_rl_repo"):
    if _p not in sys.path:
        sys.path.insert(0, _p)

import numpy as np

import concourse.bass as bass
import concourse.mybir as mybir
from concourse import bacc
from concourse.tile import TileContext
from concourse.masks import make_identity

F32 = mybir.dt.float32
F16 = mybir.dt.float16
I16 = mybir.dt.int16
P = 128

N_CORES = 8
B_FULL, N, D = 16, 2048, 64
B_LOC = B_FULL // N_CORES          # 2 batches per core
NT = N // P                        # 16 tiles of 128 along q and j
TQ = NT // 2                       # 8 pair-interleaved staging tiles of 256
QB = 512                           # q-block (PSUM bank width in fp32)
NQB = N // QB                      # 4 q-blocks
QTPB = QB // P                     # 4 q-tiles per q-block
GROUP = 3                          # S^T stream tiles per exp instruction
SPQ = 2 * NT                       # 32 stream tiles per q-block
GPQ = 11                           # groups per q-block: [2,3,3,...,3]

# Schraudolph constants: exp(x*0.125) ~ UNWEIGHTED sum of 2 phase-shifted
# estimates p = bc16(i0) + bc16(i0-512); the (1 + 1/sqrt2) scale and the
# optimal sawtooth offset fold into the bias (C=842), so no fp multiply is
# needed (elementwise |rel err| 1.11% vs 0.87% for the weighted average).
EXP_A = 0.125 * 1024 * 1.4426950408889634          # 184.6649...
EXP_B0 = float(15 * 1024 - 842)

# which groups (local index within q-block) use the DVE/Pool offload path
OFF_LOCAL = {0: (2, 5, 8), 1: (2, 5, 8), 2: (2, 5, 8), 3: (2, 5, 8)}
OFF_GROUPS = frozenset(qb * GPQ + g for qb, gs in OFF_LOCAL.items() for g in gs)

_nc_cache = None


def build():
    nc = bacc.Bacc(None, target_bir_lowering=False)
    q_hbm = nc.dram_tensor("queries", [B_LOC, N, D], F32, kind="ExternalInput")
    k_hbm = nc.dram_tensor("keys", [B_LOC, N, D], F32, kind="ExternalInput")
    v_hbm = nc.dram_tensor("values", [B_LOC, N, D], F32, kind="ExternalInput")
    o_hbm = nc.dram_tensor("out", [B_LOC, N, D], F32, kind="ExternalOutput")

    with TileContext(nc) as tc:
        with (
            tc.tile_pool(name="cst", bufs=1) as cst,
            tc.tile_pool(name="stage", bufs=2) as stage,
            tc.tile_pool(name="persist", bufs=1) as persist,
            tc.tile_pool(name="pt", bufs=14) as ptp,
            tc.tile_pool(name="off", bufs=2) as offp,
            tc.tile_pool(name="ost", bufs=4) as ostp,
            tc.tile_pool(name="st", bufs=2, space="PSUM") as stp,
            tc.tile_pool(name="pv", bufs=2, space="PSUM") as pvp,
        ):
            ident = cst.tile([P, P], F16)
            make_identity(nc, ident)

            # ---- persistent SBUF buffers ----
            # Q^T / K^T, batch-paired: rows 0-63 batch A (d), 64-127 batch B.
            qt = persist.tile([P, N], F16, tag="qt")
            kt = persist.tile([P, N], F16, tag="kt")
            # V' = [V | ones]: [128 j, b, t, e, 65] fp16 (pair-interleaved)
            v16 = persist.tile([P, B_LOC, TQ, 2, D + 1], F16, tag="v16")

            # Pair-interleaved staging: partition p of staged tile t holds the
            # TWO consecutive rows 256t+2p / 256t+2p+1 (e dim), so every DMA
            # descriptor is a 512B contiguous run (full bus efficiency; a 256B
            # run is charged 2x).  Row index within column-tile ct = 2t+e is a
            # fixed permutation shared by K and V (and by Q and the output
            # store), so attention math is unaffected.
            q_res = [q_hbm[b, :, :].rearrange("(t p e) d -> p t e d", p=P, e=2)
                     for b in range(B_LOC)]
            k_res = [k_hbm[b, :, :].rearrange("(t p e) d -> p t e d", p=P, e=2)
                     for b in range(B_LOC)]

            st32s, st16s = {}, {}
            for name in ("k", "q"):
                st32s[name] = stage.tile([P, TQ, B_LOC, 2, D], F32,
                                         tag=f"{name}s32", name=f"{name}s32")
                # transpose-friendly layout: for fixed (t, e) the (b, d)
                # block is one contiguous 128-wide free dim
                st16s[name] = stage.tile([P, TQ, 2, B_LOC, D], F16,
                                         tag=f"{name}s16", name=f"{name}s16")
            vs32 = stage.tile([P, B_LOC, TQ, 2, D], F32, tag="vs32")

            def load_chunk(name, t0, t1, eng, cast_eng=None):
                re_aps = k_res if name == "k" else q_res
                for b in range(B_LOC):
                    eng.dma_start(st32s[name][:, t0:t1, b],
                                  re_aps[b][:, t0:t1])
                    (cast_eng or nc.gpsimd).tensor_copy(
                        st16s[name][:, t0:t1, :, b, :],
                        st32s[name][:, t0:t1, b])

            # Loads ordered by need; first k/q chunks on the ACT queue, the
            # rest on SP so the issue paths overlap (HWDGE itself is serial).
            # Head-critical chunks cast on DVE; everything later casts on
            # the otherwise-idle Pool engine so the in-order DVE queue never
            # holds a cast that waits on late DMA data (head-of-line).
            load_chunk("q", 0, 2, nc.sync, cast_eng=nc.vector)
            load_chunk("k", 0, 2, nc.scalar, cast_eng=nc.vector)
            load_chunk("k", 2, 5, nc.scalar, cast_eng=nc.vector)
            load_chunk("k", 5, TQ, nc.scalar, cast_eng=nc.vector)
            # V: cast into v16[..., :64] (per batch-half), ones col
            nc.gpsimd.memset(v16[:, :, :, :, D:D + 1], 1.0)
            v_res = [v_hbm[b, :, :].rearrange("(t p e) d -> p t e d", p=P, e=2)
                     for b in range(B_LOC)]
            for b in range(B_LOC):
                hs = slice(0, TQ // 2)
                nc.sync.dma_start(vs32[:, b, hs], v_res[b][:, hs])
                nc.gpsimd.tensor_copy(v16[:, b, hs, :, 0:D], vs32[:, b, hs])
            load_chunk("q", 2, 5, nc.sync)
            for b in range(B_LOC):
                hs = slice(TQ // 2, TQ)
                nc.sync.dma_start(vs32[:, b, hs], v_res[b][:, hs])
                nc.gpsimd.tensor_copy(v16[:, b, hs, :, 0:D], vs32[:, b, hs])
            load_chunk("q", 5, TQ, nc.sync)

            def prep_transpose(name, ct0, dst, pool=None, n=2):
                """PE-transpose column-tiles [ct0, ct0+n) of q/k into dst.

                2n [128,64]->[64,128] transposes (n column-tiles x 2 batches)
                fill the batch-paired rows of one PSUM tile; a single DVE
                copy drains all n*128 columns to SBUF."""
                pool = pool or pvp
                tp_ps = pool.tile([P, n, P], F16,
                                  tag="st" if pool is stp else "pv",
                                  name=f"tp_{name}{ct0}")
                for i in range(n):
                    t, e = (ct0 + i) // 2, (ct0 + i) % 2
                    # one transpose of the [128, (b,d)=128] slice yields the
                    # batch-paired rows directly (b on the slower free dim)
                    nc.tensor.transpose(tp_ps[:, i, :],
                                        st16s[name][:, t, e], ident[:])
                nc.vector.tensor_copy(dst[:, ct0 * P:(ct0 + n) * P],
                                      tp_ps[:])

            # K^T tiles j0-1 now; the rest are emitted lazily inside the qb0
            # loop (pool allocations are FIFO in emission order, so a transpose
            # emitted before its staging data lands would stall the slot ring).
            # ---- main loop ----
            # stream of S^T tiles: s = qb*32 + 2*j + b
            # groups per qb: [2, 3, 3, ..., 3] (first exp after one j-tile)
            st_tiles = {}      # group -> psum tile
            pt_tiles = {}      # group -> sbuf fp16 tile

            def group_of(s):
                qb_, sl = divmod(s, SPQ)
                if qb_ == NQB - 1:
                    return qb_ * GPQ + min(sl // 3, GPQ - 1), \
                        sl % 3 if sl < 30 else sl - 30
                if sl < 2:
                    return qb_ * GPQ, sl
                return qb_ * GPQ + 1 + (sl - 2) // GROUP, (sl - 2) % GROUP

            def tiles_in_group(g):
                if g // GPQ == NQB - 1:
                    return 2 if g % GPQ == GPQ - 1 else GROUP
                return 2 if g % GPQ == 0 else GROUP

            off_i0 = {}

            def maybe_exp(g):
                """emit exp for group g once all its stream tiles are written.

                ACT path is one instruction.  The offload path emits only the
                PSUM-reading TS here (frees the S^T slot ring); the remaining
                three cheap SBUF ops are emitted ~2 j-steps later via
                finish_off so the in-order DVE queue interleaves other work
                and the chain latency hides under the PV lag."""
                w = tiles_in_group(g) * QB
                if g in OFF_GROUPS:
                    i0 = offp.tile([P, GROUP * QB], I16, tag="i0",
                                   name=f"i0g{g}")
                    nc.vector.tensor_scalar(i0[:, :w], st_tiles[g][:, :w],
                                            EXP_A, EXP_B0,
                                            mybir.AluOpType.mult,
                                            mybir.AluOpType.add)
                    off_i0[g] = i0
                    return
                pt_t = ptp.tile([P, GROUP * QB], F16, tag="pt", name=f"ptg{g}")
                nc.scalar.activation(
                    pt_t[:, :w], st_tiles[g][:, :w],
                    mybir.ActivationFunctionType.Exp, scale=0.125,
                )
                pt_tiles[g] = pt_t

            def finish_off(g):
                w = tiles_in_group(g) * QB
                i0 = off_i0.pop(g)
                pt_t = ptp.tile([P, GROUP * QB], F16, tag="pt", name=f"ptg{g}")
                i1 = offp.tile([P, GROUP * QB], I16, tag="i1", name=f"i1g{g}")
                nc.vector.tensor_scalar(i1[:, :w], i0[:, :w], 512, None,
                                        mybir.AluOpType.subtract)
                nc.vector.tensor_tensor(pt_t[:, :w], i1[:, :w].bitcast(F16),
                                        i0[:, :w].bitcast(F16),
                                        mybir.AluOpType.add)
                pt_tiles[g] = pt_t

            def pt_slice(s, t4):
                """lhsT slice [128 j, 128 q] for stream tile s, q-subtile t4"""
                g, slot = group_of(s)
                c0 = slot * QB + t4 * P
                return pt_tiles[g][:, c0:c0 + P]

            # Q^T tiles for q-block 0
            prep_transpose("q", 0, qt, pool=pvp)
            prep_transpose("q", 2, qt, pool=stp)
            prep_transpose("k", 0, kt, pool=pvp)

            # ---- per-q-block loop ----
            o_acc = {}
            for qb in range(NQB):
                qs = slice(qb * QB, (qb + 1) * QB)
                PV_LAG = 14 if qb == 0 else 7
                pending_off = {}
                for step in range(NT + PV_LAG):
                    if step < NT:
                        j = step
                        if qb == 0 and j in (0, 8):
                            prep_transpose("k", j + 2, kt,
                                           n=(8 if j == 0 else 6))
                        if qb + 1 < NQB and j == 3:
                            prep_transpose("q", (qb + 1) * QTPB, qt, n=4)
                        for b in range(B_LOC):
                            s = qb * SPQ + 2 * j + b
                            g, slot = group_of(s)
                            if slot == 0:
                                st_tiles[g] = stp.tile([P, GROUP * QB], F32,
                                                       tag="st", name=f"stg{g}")
                            rows = slice(b * D, (b + 1) * D)
                            nc.tensor.matmul(
                                st_tiles[g][:, slot * QB:(slot + 1) * QB],
                                kt[rows, j * P:(j + 1) * P],
                                qt[rows, qs],
                                start=True, stop=True,
                            )
                            if slot == tiles_in_group(g) - 1:
                                maybe_exp(g)
                                if g in OFF_GROUPS:
                                    pending_off.setdefault(step + 2,
                                                           []).append(g)
                    for g in pending_off.pop(step, ()):
                        finish_off(g)
                    if step == PV_LAG:
                        for b in range(B_LOC):
                            o_acc[b] = pvp.tile([P, QTPB, D + 1], F32, tag="pv",
                                                name=f"oacc{qb}_{b}")
                    if step >= PV_LAG and step - PV_LAG < NT:
                        j = step - PV_LAG
                        for b in range(B_LOC):
                            s = qb * SPQ + 2 * j + b
                            for t4 in range(QTPB):
                                # start=True zeroes the ENTIRE psum bank, so
                                # only the first matmul into this o_acc bank
                                # may set it; later regions accumulate onto
                                # the already-zeroed bank.
                                nc.tensor.matmul(
                                    o_acc[b][:, t4, :],
                                    pt_slice(s, t4),
                                    v16[:, b, j // 2, j % 2, :],
                                    start=(j == 0 and t4 == 0),
                                    stop=(j == NT - 1),
                                    skip_group_check=True,
                                )
                for b in range(B_LOC):
                    recip4 = ostp.tile([P, QTPB, 1], F32, tag="recip",
                                       name=f"recip{qb}_{b}")
                    nc.vector.reciprocal(recip4[:], o_acc[b][:, :, D:D + 1])
                    o_out = ostp.tile([P, QTPB, D], F32, tag="oo",
                                      name=f"oo{qb}_{b}")
                    nc.vector.tensor_tensor(
                        o_out[:], o_acc[b][:, :, 0:D],
                        recip4[:].to_broadcast((P, QTPB, D)),
                        mybir.AluOpType.mult,
                    )
                    o_dst = o_hbm[b, :, :].rearrange(
                        "(t p e) d -> p t e d", p=P,
                        e=2)[:, 2 * qb:2 * qb + 2]
                    nc.sync.dma_start(o_dst, o_out[:])

    nc.compile()
    return nc


def get_nc():
    global _nc_cache
    if _nc_cache is None:
        _nc_cache = build()
    return _nc_cache


def kernel(queries: np.ndarray, keys: np.ndarray, values: np.ndarray) -> np.ndarray:
    from concourse.bass_utils import run_bass_kernel_spmd

    queries = np.ascontiguousarray(np.asarray(queries, dtype=np.float32))
    keys = np.ascontiguousarray(np.asarray(keys, dtype=np.float32))
    values = np.ascontiguousarray(np.asarray(values, dtype=np.float32))

    nc = get_nc()
    in_maps = []
    for c in range(N_CORES):
        sl = slice(c * B_LOC, (c + 1) * B_LOC)
        in_maps.append({
            "queries": queries[sl],
            "keys": keys[sl],
            "values": values[sl],
        })
    res = run_bass_kernel_spmd(nc, in_maps, core_ids=list(range(N_CORES)))
    return np.concatenate([r["out"] for r in res.results], axis=0)


if __name__ == "__main__":
    rng = np.random.default_rng(0)
    q = rng.standard_normal((B_FULL, N, D), dtype=np.float32)
    k = rng.standard_normal((B_FULL, N, D), dtype=np.float32)
    v = rng.standard_normal((B_FULL, N, D), dtype=np.float32)
    o = kernel(queries=q, keys=k, values=v)
    s = q @ k.transpose(0, 2, 1) / np.sqrt(D)
    w = np.exp(s - s.max(-1, keepdims=True))
    w /= w.sum(-1, keepdims=True)
    ref = w @ v
    err = np.abs(o - ref).max() / np.abs(ref).max()
    print("rel err:", err)
